# revision 1
# baseline (speedup 1.0000x reference)
"""NNConv (gnn_message_passing) SPMD kernel for 8 trn2 NeuronCores.

Strategy (dst-sharded, both layers):
  - Each core owns a contiguous range of NLOC nodes (dst sharding). Edges are
    assigned to the core owning their dst.
  - msg = kron([1, ea], h_src) @ Wstack  (the NNConv per-edge weight matmul
    factorizes into a plain matmul over a 128-wide feature built from
    c = [1, ea0, ea1, ea2] outer h_src).
  - h_src gather: DMA transpose-gather (256B rows, bf16, replicated 4x) from a
    table; layer 1 gathers from an x-table in DRAM (host-built input), layer 2
    gathers from an SBUF-resident table built from the allgathered compact h1.
  - Aggregation (segment sum over dst): one-hot PE matmuls into PSUM-resident
    per-window accumulators (window = 128 dst nodes), fused with the root-term
    matmul and ReLU.
  - One AllGather (compact h1, bf16) between the layers.
  - Edge layout is made identical across cores via a shared R-table
    (cell (src-quarter, dst-window) padded to the max count over cores), so a
    single SPMD program works for all 8 cores.
"""

import sys

if '/opt/trn_rl_repo' not in sys.path:
    sys.path.insert(0, '/opt/trn_rl_repo')

from contextlib import ExitStack

import ml_dtypes
import numpy as np

import concourse.bacc as bacc
import concourse.bass as bass
from concourse import mybir, tile
from concourse.bass_utils import run_bass_kernel_spmd
from concourse import library_config

BF16 = ml_dtypes.bfloat16
AF = mybir.ActivationFunctionType
ALU = mybir.AluOpType

FULL_CFG = dict(N=100000, E=400000, W=8, DIM=3, HID=32)


def _ceil(a, b):
    return -(-a // b) * b


def make_geom(N, W):
    NLOC = N // W
    NLOCP = _ceil(NLOC, 128)
    NP = W * NLOCP
    assert NP % 4 == 0
    QS = NP // 4          # table rows per src-quarter
    assert QS % 128 == 0
    NW = NLOCP // 128     # dst windows per core
    return NLOC, NLOCP, NP, QS, NW


def wrap_idx16(idx):
    """Edge i -> [i%16, i//16], tiled to 128 partitions (int16)."""
    a = np.asarray(idx, np.int16).reshape(-1, 16).T
    return np.tile(a, (8, 1))


def host_prep(x, edge_index, edge_attr, params, cfg):
    """Build per-core input arrays + shared structural metadata."""
    N, E, W, DIM, HID = cfg['N'], cfg['E'], cfg['W'], cfg['DIM'], cfg['HID']
    NLOC, NLOCP, NP, QS, NW = make_geom(N, W)

    src = np.asarray(edge_index[0], np.int64)
    dst = np.asarray(edge_index[1], np.int64)
    ea = np.asarray(edge_attr, np.float32)

    tr = (src // NLOC) * NLOCP + (src % NLOC)    # gather-table row
    core = dst // NLOC
    q = tr // QS
    dl = dst % NLOC                              # dst local id
    w = dl // 128                                # dst window

    # --- shared cell table: R[q, w] = max over cores of cell count ---------
    key = (core * 4 + q) * NW + w
    cnt = np.bincount(key, minlength=W * 4 * NW).reshape(W, 4, NW)
    R = cnt.max(axis=0)                          # [4, NW]
    # pad each quarter's total to a multiple of 512 (extend last cell)
    for qq in range(4):
        tot = int(R[qq].sum())
        R[qq, NW - 1] += _ceil(tot, 512) - tot
    qsz = R.sum(axis=1)                          # [4] padded quarter sizes
    qoff = np.concatenate([[0], np.cumsum(qsz)])[:4 + 0 + 1]
    EP = int(qsz.sum())
    CH = EP // 128                               # chunks
    TB = EP // 512                               # 512-edge blocks

    # cell offsets in the edge layout (shared across cores)
    coff = np.zeros((4, NW), np.int64)
    run = 0
    cell_list = []                               # (q, w, off, len) in order
    for qq in range(4):
        for ww in range(NW):
            coff[qq, ww] = run
            cell_list.append((qq, ww, run, int(R[qq, ww])))
            run += int(R[qq, ww])
    assert run == EP

    # --- per-chunk pair metadata (shared) ---------------------------------
    pairs = [[] for _ in range(CH)]              # chunk -> [(w, paircol)]
    npairs = 0
    for (qq, ww, off, ln) in cell_list:
        if ln == 0:
            continue
        k0, k1 = off // 128, (off + ln - 1) // 128
        for k in range(k0, k1 + 1):
            pairs[k].append((ww, npairs))
            npairs += 1
    # last touch per window for matmul stop flags
    last_touch = {}                              # w -> (chunk, paircol) or None
    for k in range(CH):
        for (ww, col) in pairs[k]:
            last_touch[ww] = (k, col)

    # gather calls per quarter: pieces of <=2048, all 512-multiples
    calls = []                                   # (q, edge_off, size)
    for qq in range(4):
        o = 0
        while o < qsz[qq]:
            s = min(2048, int(qsz[qq]) - o)
            calls.append((qq, int(qoff[qq]) + o, s))
            o += s

    # --- per-core arrays ---------------------------------------------------
    order = np.lexsort((src, dl, w, q, core))
    gidx = np.zeros((W, EP), np.int64)
    cvals = np.zeros((W, 4, EP), np.float32)
    dlv = np.full((W, EP), -10000.0, np.float32)

    so_src = src[order]
    so_tr = tr[order]
    so_core = core[order]
    so_q = q[order]
    so_w = w[order]
    so_dl = dl[order]
    so_ea = ea[order]

    # slot assignment: edges are already grouped by (core, q, w) and sorted by
    # dst within each cell; place each (core,q,w) group at the shared cell off.
    ckey = (so_core * 4 + so_q) * NW + so_w
    # start index of each group in the sorted arrays
    grp_starts = np.flatnonzero(np.r_[True, ckey[1:] != ckey[:-1]])
    grp_ends = np.r_[grp_starts[1:], len(ckey)]
    for gs, ge in zip(grp_starts, grp_ends):
        c = int(so_core[gs]); qq = int(so_q[gs]); ww = int(so_w[gs])
        o = int(coff[qq, ww])
        n = ge - gs
        gidx[c, o:o + n] = so_tr[gs:ge] - qq * QS
        cvals[c, 0, o:o + n] = 1.0
        cvals[c, 1:4, o:o + n] = so_ea[gs:ge].T
        dlv[c, o:o + n] = so_dl[gs:ge].astype(np.float32)

    # wrapped gather idx [W, 128, EP//16]
    gidx16 = np.stack([wrap_idx16(gidx[c]) for c in range(W)])

    ct4 = cvals.astype(BF16)                     # [W, 4, EP]

    # dstshift [W, 128, npairs]: for pair (k, w): dl - 128*w per chunk slot
    # slot layout within chunk: edge j of chunk k sits at position 128k+j and
    # maps to onehot partition j.
    dsf = np.zeros((W, 128, max(npairs, 1)), np.float32)
    for k in range(CH):
        for (ww, col) in pairs[k]:
            for c in range(W):
                dsf[c, :, col] = dlv[c, k * 128:(k + 1) * 128] - 128.0 * ww

    # x4 gather table [NP, 128] bf16: [x(3)|0]*4, zero pad rows
    x = np.asarray(x, np.float32)
    x4 = np.zeros((NP, 128), np.float32)
    rows = np.arange(NP)
    rc, rl = rows // NLOCP, rows % NLOCP
    valid = rl < NLOC
    nid = np.clip(rc * NLOC + rl, 0, N - 1)
    for d in range(4):
        x4[valid, 32 * d:32 * d + DIM] = x[nid[valid]]
    x4 = x4.astype(BF16)

    # x_augT packed (per core): window w at [32*(w%3):+4, (w//3)*128:+128]
    XCOLS = _ceil(NW, 3) // 3 * 128
    xaug = np.zeros((W, 128, XCOLS), np.float32)
    for c in range(W):
        xa = np.zeros((4, NLOCP), np.float32)
        xa[:DIM, :NLOC] = x[c * NLOC:(c + 1) * NLOC].T
        xa[3, :NLOC] = 1.0
        for ww in range(NW):
            xaug[c, 32 * (ww % 3):32 * (ww % 3) + 4,
                 (ww // 3) * 128:(ww // 3) * 128 + 128] = \
                xa[:, ww * 128:(ww + 1) * 128]

    # weights
    def stack_w(Wn, bn, in_c):
        S = np.zeros((128, HID), np.float32)
        B = bn.reshape(in_c, HID)
        S[0:in_c] = B
        for d in range(3):
            S[32 * (d + 1):32 * (d + 1) + in_c] = Wn[d].reshape(in_c, HID)
        return S.astype(BF16)

    w1stack = stack_w(np.asarray(params['Wn1'], np.float32),
                      np.asarray(params['bn1'], np.float32), DIM)
    w2stack = stack_w(np.asarray(params['Wn2'], np.float32),
                      np.asarray(params['bn2'], np.float32), HID)
    expand4 = np.zeros((4, 128), np.float32)
    for d in range(4):
        expand4[d, 32 * d:32 * d + 32] = 1.0
    expand4 = expand4.astype(BF16)
    root1a = np.concatenate([np.asarray(params['root1'], np.float32),
                             np.asarray(params['b1'], np.float32)[None]], 0)
    root1a_p = np.zeros((68, HID), np.float32)
    for g in range(3):
        root1a_p[32 * g:32 * g + DIM] = root1a[:DIM]
        root1a_p[32 * g + 3] = root1a[DIM]
    # (3 replicas at base partitions 0/32/64 to match xaug group bases)
    root2a = np.concatenate([np.asarray(params['root2'], np.float32),
                             np.asarray(params['b2'], np.float32)[None]],
                            0).astype(BF16)
    wf1a = np.concatenate([np.asarray(params['Wf1'], np.float32),
                           np.asarray(params['bf1'], np.float32)[None]], 0)
    wf2a = np.concatenate([np.asarray(params['Wf2'], np.float32),
                           np.asarray(params['bf2'], np.float32)[None]], 0)
    iota = np.tile(np.arange(128, dtype=np.float32)[None, :],
                   (128, 1)).astype(BF16)
    ident = np.eye(128, dtype=np.float32)

    meta = dict(NLOC=NLOC, NLOCP=NLOCP, NP=NP, QS=QS, NW=NW, EP=EP, CH=CH,
                TB=TB, XCOLS=XCOLS, npairs=npairs, pairs=pairs,
                last_touch=last_touch, calls=calls, qoff=[int(v) for v in qoff],
                qsz=[int(v) for v in qsz], W=W, HID=HID, DIM=DIM)

    shared = dict(x4tab=x4, w1stack=w1stack, w2stack=w2stack, expand4=expand4,
                  root1a=root1a_p, root2a=root2a, wf1a=wf1a, wf2a=wf2a,
                  iota=iota, ident=ident)
    in_maps = []
    for c in range(W):
        m = dict(shared)
        m['gidx'] = gidx16[c]
        m['ct4'] = ct4[c]
        m['dsf'] = dsf[c]
        m['xaug'] = xaug[c]
        in_maps.append(m)
    return in_maps, meta


def build_bass(meta):
    W, HID = meta['W'], meta['HID']
    NLOCP, NP, QS, NW = meta['NLOCP'], meta['NP'], meta['QS'], meta['NW']
    EP, CH, TB = meta['EP'], meta['CH'], meta['TB']
    XCOLS, npairs = meta['XCOLS'], meta['npairs']
    pairs, calls = meta['pairs'], meta['calls']
    RANKS_Q = QS // 128
    f32, bf16, i16 = mybir.dt.float32, mybir.dt.bfloat16, mybir.dt.int16

    nc = bacc.Bacc("TRN2", target_bir_lowering=False, debug=False,
                   num_devices=W, enable_asserts=False)

    # I/O ------------------------------------------------------------------
    x4tab = nc.dram_tensor("x4tab", [NP, 128], bf16, kind="ExternalInput")
    gidx_d = nc.dram_tensor("gidx", [128, EP // 16], i16, kind="ExternalInput")
    ct4_d = nc.dram_tensor("ct4", [4, EP], bf16, kind="ExternalInput")
    dsf_d = nc.dram_tensor("dsf", [128, max(npairs, 1)], f32,
                           kind="ExternalInput")
    xaug_d = nc.dram_tensor("xaug", [128, XCOLS], f32, kind="ExternalInput")
    w1_d = nc.dram_tensor("w1stack", [128, HID], bf16, kind="ExternalInput")
    w2_d = nc.dram_tensor("w2stack", [128, HID], bf16, kind="ExternalInput")
    ex4_d = nc.dram_tensor("expand4", [4, 128], bf16, kind="ExternalInput")
    r1_d = nc.dram_tensor("root1a", [68, HID], f32, kind="ExternalInput")
    r2_d = nc.dram_tensor("root2a", [33, HID], bf16, kind="ExternalInput")
    wf1_d = nc.dram_tensor("wf1a", [33, HID], f32, kind="ExternalInput")
    wf2_d = nc.dram_tensor("wf2a", [33, 1], f32, kind="ExternalInput")
    iota_d = nc.dram_tensor("iota", [128, 128], bf16, kind="ExternalInput")
    id_d = nc.dram_tensor("ident", [128, 128], f32, kind="ExternalInput")
    out_d = nc.dram_tensor("out", [1, NLOCP], f32, kind="ExternalOutput")

    cc_in = nc.dram_tensor("cc_in", [NLOCP, HID], bf16)
    cc_out = nc.dram_tensor("cc_out", [NP, HID], bf16, addr_space="Shared")

    ctx = ExitStack()
    with tile.TileContext(nc) as tc:
      with ctx:
        const = ctx.enter_context(tc.tile_pool(name="const", bufs=1))
        big = ctx.enter_context(tc.tile_pool(name="big", bufs=1))
        pipe = ctx.enter_context(tc.tile_pool(name="pipe", bufs=2))
        ohp = ctx.enter_context(tc.tile_pool(name="ohp", bufs=3))
        tabp = ctx.enter_context(tc.tile_pool(name="tabp", bufs=1))

        nc.gpsimd.load_library(library_config.mlp)

        # ---- constant loads ----
        def load(pool, dram, shape, dtype):
            t = pool.tile(shape, dtype, tag="c_" + dram.name)
            nc.sync.dma_start(out=t[:], in_=dram[:, :])
            return t

        gidx_s = load(const, gidx_d, [128, EP // 16], i16)
        dsf_s = load(const, dsf_d, [128, max(npairs, 1)], f32)
        xaug_s = load(const, xaug_d, [128, XCOLS], f32)
        w1_s = load(const, w1_d, [128, HID], bf16)
        w2_s = load(const, w2_d, [128, HID], bf16)
        ex4_s = load(const, ex4_d, [4, 128], bf16)
        r1_s = load(const, r1_d, [68, HID], f32)
        r2_s = load(const, r2_d, [33, HID], bf16)
        wf1_s = load(const, wf1_d, [33, HID], f32)
        wf2_s = load(const, wf2_d, [33, 1], f32)
        iota_s = load(const, iota_d, [128, 128], bf16)
        id_s = load(const, id_d, [128, 128], f32)

        msg_s = big.tile([128, CH * 32], bf16)      # all msgs of one layer
        h1c_s = big.tile([128, NW * 32], bf16)      # compact local h1
        h1T_s = big.tile([33, NLOCP], bf16)         # h1^T augmented
        nc.vector.memset(h1T_s[32:33, :], 1.0)

        bywin = [[] for _ in range(NW)]
        for k in range(CH):
            for (ww, col) in pairs[k]:
                bywin[ww].append((k, col))

        def do_block(ps1, t, Gt, b, wstack):
            """One 512-edge block of pass 1."""
            ct = pipe.tile([4, 2048], bf16, tag="ct")
            nc.sync.dma_start(out=ct[:, 0:512],
                              in_=ct4_d[:, t * 512:(t + 1) * 512])
            Cp = ps1.tile([128, 512], f32, tag="C")
            nc.tensor.matmul(Cp[:], ex4_s[:], ct[:, 0:512],
                             start=True, stop=True)
            Ft = pipe.tile([128, 512], bf16, tag="F")
            nc.vector.tensor_tensor(Ft[:], Gt[:, 512 * b:512 * b + 512],
                                    Cp[:], ALU.mult)
            Mp = ps1.tile([128, 128], f32, tag="M")
            for j in range(4):
                nc.tensor.matmul(Mp[:, 32 * j:32 * j + 32],
                                 Ft[:, 128 * j:128 * j + 128],
                                 wstack[:], start=True, stop=True)
            nc.scalar.activation(msg_s[:, t * 128:(t + 1) * 128],
                                 Mp[:], AF.Copy)

        def pass2(layer):
            """Segment-sum + root + relu (+ fused fc on layer 2), in two
            window-halves to bound PSUM usage."""
            HWN = (NW + 1) // 2
            for half in range(2):
                wlo = half * HWN
                whi = min(NW, (half + 1) * HWN)
                if wlo >= whi:
                    continue
                with tc.tile_pool(name="ps2", bufs=1, space="PSUM") as ps2:
                    aggp = ps2.tile([128, (whi - wlo) * 32], f32, tag="agg")
                    for ww in range(wlo, whi):
                        if layer == 1:
                            g = 32 * (ww % 3)
                            lhs = xaug_s[g:g + 4,
                                         (ww // 3) * 128:(ww // 3) * 128 + 128]
                            rr = r1_s[g:g + 4, :]
                        else:
                            lhs = h1T_s[:, ww * 128:(ww + 1) * 128]
                            rr = r2_s[:]
                        a = aggp[:, 32 * (ww - wlo):32 * (ww - wlo) + 32]
                        nc.tensor.matmul(a, lhs, rr, start=True,
                                         stop=(len(bywin[ww]) == 0),
                                         skip_group_check=True)
                        for pi, (k, col) in enumerate(bywin[ww]):
                            oh = ohp.tile([128, 128], bf16, tag="oh")
                            nc.vector.tensor_scalar(oh[:], iota_s[:],
                                                    dsf_s[:, col:col + 1], 0.0,
                                                    ALU.is_equal, ALU.bypass)
                            nc.tensor.matmul(a, oh[:],
                                             msg_s[:, 32 * k:32 * k + 32],
                                             start=False,
                                             stop=(pi == len(bywin[ww]) - 1),
                                             skip_group_check=True)
                    # relu (+ transpose to hT; + fc on layer 2)
                    for w0 in range(wlo, whi, 4):
                        nsub = min(4, whi - w0)
                        trp = ps2.tile([32, 512], f32, tag="tr")
                        for i in range(nsub):
                            ww = w0 + i
                            a = aggp[:, 32 * (ww - wlo):32 * (ww - wlo) + 32]
                            if layer == 1:
                                nc.scalar.activation(
                                    h1c_s[:, 32 * ww:32 * ww + 32], a, AF.Relu)
                            hf = pipe.tile([128, 32], f32, tag="hf")
                            nc.scalar.activation(hf[:], a, AF.Relu)
                            nc.tensor.transpose(trp[:, 128 * i:128 * i + 128],
                                                hf[:], id_s[:])
                        span = 128 * nsub
                        if layer == 1:
                            nc.scalar.activation(
                                h1T_s[0:32, 128 * w0:128 * w0 + span],
                                trp[:, 0:span], AF.Copy)
                        else:
                            h2t = pipe.tile([33, 512], f32, tag="h2t")
                            nc.scalar.activation(h2t[0:32, 0:span],
                                                 trp[:, 0:span], AF.Copy)
                            nc.vector.memset(h2t[32:33, 0:span], 1.0)
                            f1 = ps2.tile([32, 512], f32, tag="f1")
                            nc.tensor.matmul(f1[:, 0:span], wf1_s[:],
                                             h2t[:, 0:span],
                                             start=True, stop=True)
                            h3t = pipe.tile([33, 512], f32, tag="h3t")
                            nc.scalar.activation(h3t[0:32, 0:span],
                                                 f1[:, 0:span], AF.Relu)
                            nc.vector.memset(h3t[32:33, 0:span], 1.0)
                            f2 = ps2.tile([1, 512], f32, tag="f2")
                            nc.tensor.matmul(f2[:, 0:span], wf2_s[:],
                                             h3t[:, 0:span],
                                             start=True, stop=True)
                            ot = pipe.tile([1, 512], f32, tag="ot")
                            nc.scalar.activation(ot[:, 0:span], f2[:, 0:span],
                                                 AF.Copy)
                            nc.sync.dma_start(
                                out=out_d[:, 128 * w0:128 * w0 + span],
                                in_=ot[:, 0:span])

        # ================= layer 1 =================
        with nc.named_scope("l1_pass1"), \
             tc.tile_pool(name="ps1", bufs=2, space="PSUM") as ps1:
            for (qq, eoff, csz) in calls:
                Gt = pipe.tile([128, 2048], bf16, tag="G")
                g3 = Gt[:, 0:csz].rearrange("p (o n) -> p o n", o=1)
                nc.gpsimd.dma_gather(
                    g3, x4tab[qq * QS:(qq + 1) * QS, :],
                    gidx_s[:, eoff // 16:(eoff + csz) // 16],
                    csz, csz, 128, transpose=True, single_packet=False)
                for b in range(csz // 512):
                    do_block(ps1, (eoff + b * 512) // 512, Gt, b, w1_s)
        with nc.named_scope("l1_pass2"):
            pass2(1)

        # ship compact h1, allgather
        nc.sync.dma_start(
            out=cc_in.ap().rearrange("(w p) h -> p w h", p=128),
            in_=h1c_s[:].rearrange("p (w h) -> p w h", h=HID))
        with nc.named_scope("allgather"):
            nc.gpsimd.collective_compute(
                "AllGather", ALU.bypass, replica_groups=[list(range(W))],
                ins=[cc_in.ap().opt()], outs=[cc_out.ap().opt()])

        # ================= layer 2 =================
        with nc.named_scope("l2_pass1"), \
             tc.tile_pool(name="ps1b", bufs=2, space="PSUM") as ps1b:
            for qq in range(4):
                cw = tabp.tile([128, RANKS_Q * 32], bf16, tag="cw")
                nc.sync.dma_start(
                    out=cw[:].rearrange("p (r h) -> p r h", h=HID),
                    in_=cc_out.ap().rearrange("(q r p) h -> q p r h",
                                              q=4, p=128)[qq])
                tq = tabp.tile([128, RANKS_Q * 128], bf16, tag="tq")
                tq4 = tq[:].rearrange("p (r d h) -> p r d h", d=4, h=HID)
                cw3 = cw[:].rearrange("p (r h) -> p r h", h=HID)
                for d in range(4):
                    nc.vector.tensor_copy(tq4[:, :, d, :], cw3)
                for (cq, eoff, csz) in calls:
                    if cq != qq:
                        continue
                    Gt = pipe.tile([128, 2048], bf16, tag="G")
                    g3 = Gt[:, 0:csz].rearrange("p (o n) -> p o n", o=1)
                    nc.gpsimd.dma_gather(
                        g3, tq[:], gidx_s[:, eoff // 16:(eoff + csz) // 16],
                        csz, csz, 128, transpose=True, single_packet=False,
                        sbuf_tokens_per_rank=128, sbuf_free_dim_per_rank=256,
                        sbuf_free_dim_pad_per_rank=0, sbuf_byte_offset=0)
                    for b in range(csz // 512):
                        do_block(ps1b, (eoff + b * 512) // 512, Gt, b, w2_s)
        with nc.named_scope("l2_pass2"):
            pass2(2)
    return nc


def run_kernel(inputs, cfg=None, trace=False):
    cfg = cfg or FULL_CFG
    W = cfg['W']
    params = {k: inputs[k] for k in
              ('Wn1', 'bn1', 'root1', 'b1', 'Wn2', 'bn2', 'root2', 'b2',
               'Wf1', 'bf1', 'Wf2', 'bf2')}
    in_maps, meta = host_prep(inputs['x'], inputs['edge_index'],
                              inputs['edge_attr'], params, cfg)
    nc = build_bass(meta)
    nc.finalize()
    res = run_bass_kernel_spmd(nc, in_maps, core_ids=list(range(W)),
                               trace=trace)
    NLOC = meta['NLOC']
    out = np.zeros((cfg['N'], 1), np.float32)
    for c in range(W):
        out[c * NLOC:(c + 1) * NLOC, 0] = res.results[c]['out'][0, :NLOC]
    return out, res


def kernel(**inputs):
    out, _ = run_kernel(inputs)
    return out



# revision 17
# speedup vs baseline: 1.3599x; 1.3599x over previous
"""NNConv (gnn_message_passing) SPMD kernel for 8 trn2 NeuronCores.

Strategy (dst-sharded, both layers):
  - Each core owns a contiguous range of NLOC nodes (dst sharding). Edges are
    assigned to the core owning their dst, and laid out half-major:
    (dst-window-half, src-quarter, dst-window, dst).
  - Layer 1 needs NO on-device gather: the host ships x[src] and the edge
    coefficients c=[1,ea] replicated into a 16-row outer-product layout
    (pure indexing / replication); one DVE multiply forms F = c (x) x_src and
    one small matmul per 128-edge chunk produces the messages.
  - Layer 2 gathers h1[src] via the SBUF transpose-gather (Q7 SWDGE). All
    compute (messages, one-hot aggregation, root terms, fc) is interleaved
    with the gather stream so the Q7 descriptor generation is the only wall.
  - Aggregation (segment sum over dst): one-hot PE matmuls into a PSUM
    accumulator holding one half of the dst windows (4 banks), accumulated
    across all 4 src-quarters, fused with the root-term matmul and ReLU.
  - One AllGather (compact h1, bf16) between the layers.
  - Edge layout is made identical across cores via a shared R-table
    (cell (half, src-quarter, dst-window) padded to the max count over cores).
"""

import sys

if '/opt/trn_rl_repo' not in sys.path:
    sys.path.insert(0, '/opt/trn_rl_repo')

from contextlib import ExitStack

import ml_dtypes
import numpy as np

import concourse.bacc as bacc
import concourse.bass as bass
from concourse import mybir, tile
from concourse.bass_utils import run_bass_kernel_spmd
from concourse import library_config

BF16 = ml_dtypes.bfloat16
AF = mybir.ActivationFunctionType
ALU = mybir.AluOpType

FULL_CFG = dict(N=100000, E=400000, W=8, DIM=3, HID=32)


def _ceil(a, b):
    return -(-a // b) * b


def make_geom(N, W):
    NLOC = N // W
    NLOCP = _ceil(NLOC, 128)
    NP = W * NLOCP
    assert NP % 4 == 0
    QS = NP // 4          # table rows per src-quarter
    assert QS % 128 == 0
    NW = NLOCP // 128     # dst windows per core
    return NLOC, NLOCP, NP, QS, NW


def wrap_idx16(idx):
    """Edge i -> [i%16, i//16], tiled to 128 partitions (int16)."""
    a = np.asarray(idx, np.int16).reshape(-1, 16).T
    return np.tile(a, (8, 1))


def host_prep(x, edge_index, edge_attr, params, cfg):
    """Build per-core input arrays + shared structural metadata."""
    N, E, W, DIM, HID = cfg['N'], cfg['E'], cfg['W'], cfg['DIM'], cfg['HID']
    NLOC, NLOCP, NP, QS, NW = make_geom(N, W)
    HWN = (NW + 1) // 2                         # windows per half

    src = np.asarray(edge_index[0], np.int64)
    dst = np.asarray(edge_index[1], np.int64)
    ea = np.asarray(edge_attr, np.float32)
    x = np.asarray(x, np.float32)

    tr = (src // NLOC) * NLOCP + (src % NLOC)    # gather-table row
    core = dst // NLOC
    q = tr // QS
    dl = dst % NLOC                              # dst local id
    w = dl // 128                                # dst window
    hf = (w >= HWN).astype(np.int64)             # dst-window half

    # --- shared cell table: R[h, q, w'] = max over cores of cell count -----
    # cells ordered half-major: (h, q, w within half)
    win_in_half = w - hf * HWN
    NWH = [HWN, NW - HWN]                        # windows per half
    key = ((core * 2 + hf) * 4 + q) * HWN + win_in_half
    cnt = np.bincount(key, minlength=W * 2 * 4 * HWN).reshape(W, 2, 4, HWN)
    R = cnt.max(axis=0)                          # [2, 4, HWN]
    # pad each (half, quarter) group total to a multiple of 512
    for h in range(2):
        for qq in range(4):
            tot = int(R[h, qq, :NWH[h]].sum())
            R[h, qq, NWH[h] - 1] += _ceil(max(tot, 512), 512) - tot
    gsz = np.array([[int(R[h, qq, :NWH[h]].sum()) for qq in range(4)]
                    for h in range(2)])          # [2, 4]
    goff = np.zeros((2, 4), np.int64)
    run = 0
    cell_list = []                               # (h, q, w, off, len) in order
    coff = {}
    for h in range(2):
        for qq in range(4):
            goff[h, qq] = run
            for ww in range(NWH[h]):
                wglob = h * HWN + ww
                coff[(h, qq, wglob)] = run
                cell_list.append((h, qq, wglob, run, int(R[h, qq, ww])))
                run += int(R[h, qq, ww])
    EP = run
    assert EP % 512 == 0
    CH = EP // 128                               # chunks

    # --- per-chunk pair metadata (shared) ---------------------------------
    pairs = [[] for _ in range(CH)]              # chunk -> [(wglob, paircol)]
    npairs = 0
    for (h, qq, wglob, off, ln) in cell_list:
        if ln == 0:
            continue
        k0, k1 = off // 128, (off + ln - 1) // 128
        for k in range(k0, k1 + 1):
            pairs[k].append((wglob, npairs))
            npairs += 1
    # cells grouped per (h, q) for emission: [(wglob, [(k, col), ...]), ...]
    cells_hq = {(h, qq): [] for h in range(2) for qq in range(4)}
    for (h, qq, wglob, off, ln) in cell_list:
        plist = []
        if ln > 0:
            k0, k1 = off // 128, (off + ln - 1) // 128
            for k in range(k0, k1 + 1):
                col = next(c for (wv, c) in pairs[k] if wv == wglob)
                plist.append((k, col))
        cells_hq[(h, qq)].append((wglob, plist))
    # last nonempty quarter per window (for matmul stop flags)
    last_q = {}
    first_q = {}
    for h in range(2):
        for qq in range(4):
            for (wglob, plist) in cells_hq[(h, qq)]:
                if plist:
                    last_q[wglob] = qq
                    if wglob not in first_q:
                        first_q[wglob] = qq

    # gather calls per (half, quarter): pieces of <=2048, all 512-multiples
    calls = {(h, qq): [] for h in range(2) for qq in range(4)}
    for h in range(2):
        for qq in range(4):
            o = 0
            while o < gsz[h, qq]:
                s = min(2048, int(gsz[h, qq]) - o)
                calls[(h, qq)].append((int(goff[h, qq]) + o, s))
                o += s

    # --- per-core arrays ---------------------------------------------------
    order = np.lexsort((src, dl, w, q, hf, core))
    gidx = np.zeros((W, EP), np.int64)
    dlv = np.full((W, EP), -10000.0, np.float32)
    cfull = np.zeros((W, 4, EP), np.float32)     # c = [1, ea] per edge slot
    xfull = np.zeros((W, 4, EP), np.float32)     # x[src] (padded) per slot

    so_src = src[order]
    so_tr = tr[order]
    so_core = core[order]
    so_q = q[order]
    so_hf = hf[order]
    so_w = w[order]
    so_dl = dl[order]
    so_ea = ea[order]

    ckey = ((so_core * 2 + so_hf) * 4 + so_q) * NW + so_w
    grp_starts = np.flatnonzero(np.r_[True, ckey[1:] != ckey[:-1]])
    grp_ends = np.r_[grp_starts[1:], len(ckey)]
    for gs, ge in zip(grp_starts, grp_ends):
        c = int(so_core[gs]); hh = int(so_hf[gs])
        qq = int(so_q[gs]); wglob = int(so_w[gs])
        o = coff[(hh, qq, wglob)]
        n = ge - gs
        gidx[c, o:o + n] = so_tr[gs:ge] - qq * QS
        cfull[c, 0, o:o + n] = 1.0
        cfull[c, 1:4, o:o + n] = so_ea[gs:ge].T
        xfull[c, 0:3, o:o + n] = x[so_src[gs:ge]].T
        dlv[c, o:o + n] = so_dl[gs:ge].astype(np.float32)

    # wrapped gather idx [W, 128, EP//16]
    gidx16 = np.stack([wrap_idx16(gidx[c]) for c in range(W)])

    # dsf [W, 128, npairs]: dst offset within the PAIR's window (out-of-window
    # slots fall outside [0,128) and never match the iota compare)
    dsf = np.full((W, 128, max(npairs, 1)), -10000.0, np.float32)
    for k in range(CH):
        for (wglob, col) in pairs[k]:
            for c in range(W):
                dsf[c, :, col] = dlv[c, k * 128:(k + 1) * 128] - 128.0 * wglob

    # --- layer-1 no-gather tensors ----------------------------------------
    # packed [128, ceil(CH/4)*128]: chunk c -> partition group 32*(c%4),
    # columns 128*(c//4); rows within group r = 4*dc + i:
    #   ct16[r] = c_dc[e],  xs16[r] = x_i[src_e] (i<3) else 0
    CB = _ceil(CH, 4) // 4
    ct16 = np.zeros((W, 128, CB * 128), np.float32)
    xs16 = np.zeros((W, 128, CB * 128), np.float32)
    for c in range(W):
        cf = cfull[c]                            # [4, EP]
        xf = xfull[c]                            # [4, EP]
        for g in range(4):
            # chunks with c%4 == g -> columns of block c//4
            ch_ids = np.arange(g, CH, 4)
            colsrc = (ch_ids[:, None] * 128 + np.arange(128)[None, :]).ravel()
            coldst = (np.arange(len(ch_ids))[:, None] * 128 +
                      np.arange(128)[None, :]).ravel()
            for dc in range(4):
                for i in range(4):
                    r = 32 * g + 4 * dc + i
                    ct16[c, r, coldst] = cf[dc, colsrc]
                    if i < 3:
                        xs16[c, r, coldst] = xf[i, colsrc]
    ct16 = ct16.astype(BF16)
    xs16 = xs16.astype(BF16)

    # layer-2 expanded coefficients in DRAM: ct32[32g+i, e] = c_g[e]
    ct32 = np.repeat(cfull, 32, axis=1).astype(BF16)   # [W, 128, EP]

    # --- weights -----------------------------------------------------------
    Wn1 = np.asarray(params['Wn1'], np.float32)
    bn1 = np.asarray(params['bn1'], np.float32)
    Wn2 = np.asarray(params['Wn2'], np.float32)
    bn2 = np.asarray(params['bn2'], np.float32)

    # V4 for layer 1: [128, 4*HID]. Full 128-row contraction per chunk
    # (no PE tiling modes): col-block g holds V16 at rows 32g.., zeros
    # elsewhere, so chunk group g picks out only its rows.
    V16 = np.zeros((32, HID), np.float32)
    B1 = bn1.reshape(DIM, HID)
    W1r = Wn1.reshape(DIM, DIM, HID)
    for dc in range(4):
        for i in range(DIM):
            V16[4 * dc + i] = B1[i] if dc == 0 else W1r[dc - 1, i]
    V4 = np.zeros((128, 4 * HID), np.float32)
    for g in range(4):
        V4[32 * g:32 * g + 32, g * HID:(g + 1) * HID] = V16
    V4 = V4.astype(BF16)

    # w2stack for layer 2: [128, 32]
    w2stack = np.zeros((128, HID), np.float32)
    w2stack[0:HID] = bn2.reshape(HID, HID)
    for d in range(DIM):
        w2stack[32 * (d + 1):32 * (d + 1) + HID] = Wn2[d].reshape(HID, HID)
    w2stack = w2stack.astype(BF16)

    # x_augT packed (per core): window w at [32*(w%3):+4, (w//3)*128:+128]
    XCOLS = _ceil(NW, 3) // 3 * 128
    xaug = np.zeros((W, 128, XCOLS), np.float32)
    for c in range(W):
        xa = np.zeros((4, NLOCP), np.float32)
        xa[:DIM, :NLOC] = x[c * NLOC:(c + 1) * NLOC].T
        xa[3, :NLOC] = 1.0
        for ww in range(NW):
            xaug[c, 32 * (ww % 3):32 * (ww % 3) + 4,
                 (ww // 3) * 128:(ww // 3) * 128 + 128] = \
                xa[:, ww * 128:(ww + 1) * 128]

    root1a = np.concatenate([np.asarray(params['root1'], np.float32),
                             np.asarray(params['b1'], np.float32)[None]], 0)
    root1a_p = np.zeros((68, HID), np.float32)
    for g in range(3):
        root1a_p[32 * g:32 * g + DIM] = root1a[:DIM]
        root1a_p[32 * g + 3] = root1a[DIM]
    root2a = np.concatenate([np.asarray(params['root2'], np.float32),
                             np.asarray(params['b2'], np.float32)[None]],
                            0).astype(BF16)
    wf1a = np.concatenate([np.asarray(params['Wf1'], np.float32),
                           np.asarray(params['bf1'], np.float32)[None]], 0)
    wf2a = np.concatenate([np.asarray(params['Wf2'], np.float32),
                           np.asarray(params['bf2'], np.float32)[None]], 0)
    iota = np.tile(np.arange(128, dtype=np.float32)[None, :],
                   (128, 1)).astype(BF16)
    ident = np.eye(128, dtype=np.float32)

    CHH = max(int(gsz[0].sum()), int(gsz[1].sum())) // 128
    meta = dict(NLOC=NLOC, NLOCP=NLOCP, NP=NP, QS=QS, NW=NW, HWN=HWN,
                NWH=NWH, EP=EP, CH=CH, CB=CB, CHH=CHH, XCOLS=XCOLS,
                npairs=npairs,
                pairs=pairs, cells_hq=cells_hq, last_q=last_q, first_q=first_q,
                calls=calls, gsz=gsz.tolist(), goff=goff.tolist(),
                W=W, HID=HID, DIM=DIM)

    shared = dict(V4=V4, w2stack=w2stack, root1a=root1a_p, root2a=root2a,
                  wf1a=wf1a, wf2a=wf2a, iota=iota, ident=ident)
    in_maps = []
    for c in range(W):
        m = dict(shared)
        m['gidx'] = gidx16[c]
        m['dsf'] = dsf[c]
        m['xaug'] = xaug[c]
        m['ct16'] = ct16[c]
        m['xs16'] = xs16[c]
        m['ct32'] = ct32[c]
        in_maps.append(m)
    return in_maps, meta


def build_bass(meta):
    W, HID = meta['W'], meta['HID']
    NLOCP, NP, QS, NW = meta['NLOCP'], meta['NP'], meta['QS'], meta['NW']
    HWN, NWH = meta['HWN'], meta['NWH']
    EP, CH, CB, CHH = meta['EP'], meta['CH'], meta['CB'], meta['CHH']
    XCOLS, npairs = meta['XCOLS'], meta['npairs']
    pairs, calls, cells_hq = meta['pairs'], meta['calls'], meta['cells_hq']
    last_q, first_q = meta['last_q'], meta['first_q']
    RANKS_Q = QS // 128
    f32, bf16, i16 = mybir.dt.float32, mybir.dt.bfloat16, mybir.dt.int16

    nc = bacc.Bacc("TRN2", target_bir_lowering=False, debug=False,
                   num_devices=W, enable_asserts=False)

    # I/O ------------------------------------------------------------------
    gidx_d = nc.dram_tensor("gidx", [128, EP // 16], i16, kind="ExternalInput")
    dsf_d = nc.dram_tensor("dsf", [128, max(npairs, 1)], f32,
                           kind="ExternalInput")
    xaug_d = nc.dram_tensor("xaug", [128, XCOLS], f32, kind="ExternalInput")
    ct16_d = nc.dram_tensor("ct16", [128, CB * 128], bf16,
                            kind="ExternalInput")
    xs16_d = nc.dram_tensor("xs16", [128, CB * 128], bf16,
                            kind="ExternalInput")
    ct32_d = nc.dram_tensor("ct32", [128, EP], bf16, kind="ExternalInput")
    V4_d = nc.dram_tensor("V4", [128, 4 * HID], bf16, kind="ExternalInput")
    w2_d = nc.dram_tensor("w2stack", [128, HID], bf16, kind="ExternalInput")
    r1_d = nc.dram_tensor("root1a", [68, HID], f32, kind="ExternalInput")
    r2_d = nc.dram_tensor("root2a", [33, HID], bf16, kind="ExternalInput")
    wf1_d = nc.dram_tensor("wf1a", [33, HID], f32, kind="ExternalInput")
    wf2_d = nc.dram_tensor("wf2a", [33, 1], f32, kind="ExternalInput")
    iota_d = nc.dram_tensor("iota", [128, 128], bf16, kind="ExternalInput")
    id_d = nc.dram_tensor("ident", [128, 128], f32, kind="ExternalInput")
    out_d = nc.dram_tensor("out", [1, NLOCP], f32, kind="ExternalOutput")

    cc_in = nc.dram_tensor("cc_in", [NLOCP, HID], bf16)
    cc_out = nc.dram_tensor("cc_out", [NP, HID], bf16, addr_space="Shared")

    ctx = ExitStack()
    with tile.TileContext(nc) as tc:
      with ctx:
        const = ctx.enter_context(tc.tile_pool(name="const", bufs=1))
        big = ctx.enter_context(tc.tile_pool(name="big", bufs=1))
        pipe = ctx.enter_context(tc.tile_pool(name="pipe", bufs=3))
        ohp = ctx.enter_context(tc.tile_pool(name="ohp", bufs=6))

        nc.gpsimd.load_library(library_config.mlp)

        # ---- constant loads ----
        def load(pool, dram, shape, dtype):
            t = pool.tile(shape, dtype, tag="c_" + dram.name)
            nc.sync.dma_start(out=t[:], in_=dram[:, :])
            return t

        gidx_s = load(const, gidx_d, [128, EP // 16], i16)
        dsf_s = load(const, dsf_d, [128, max(npairs, 1)], f32)
        V4_s = load(const, V4_d, [128, 4 * HID], bf16)
        w2_s = load(const, w2_d, [128, HID], bf16)
        r1_s = load(const, r1_d, [68, HID], f32)
        r2_s = load(const, r2_d, [33, HID], bf16)
        wf1_s = load(const, wf1_d, [33, HID], f32)
        wf2_s = load(const, wf2_d, [33, 1], f32)
        iota_s = load(const, iota_d, [128, 128], bf16)
        id_s = load(const, id_d, [128, 128], f32)
        zer_s = const.tile([128, 512], bf16, tag="zer")
        nc.vector.memset(zer_s[:], 0.0)

        msg_s = big.tile([128, CHH * 32], bf16)     # msgs of one half
        h1c_s = big.tile([128, NW * 32], bf16)      # compact local h1
        h1T_s = big.tile([33, NLOCP], bf16)         # h1^T augmented
        nc.vector.memset(h1T_s[32:33, :], 1.0)

        def zero_agg(aggp, h):
            """Initialize the half's PSUM accumulator: one start=True matmul
            per 2KB bank (start marks the WHOLE zero-region pending-zero, so
            interleaved per-window chains must all accumulate afterwards)."""
            tot = NWH[h] * 32
            for off in range(0, tot, 512):
                wd = min(512, tot - off)
                nc.tensor.matmul(aggp[:, off:off + wd], zer_s[:, 0:128],
                                 zer_s[:, 0:wd], start=True, stop=False,
                                 skip_group_check=True)

        def agg_windows(layer, h, aggp, qq, ck0h):
            """Emit aggregation pairs for quarter qq of half h."""
            for (wglob, plist) in cells_hq[(h, qq)]:
                j = wglob - h * HWN
                a = aggp[:, 32 * j:32 * j + 32]
                if first_q.get(wglob, 0) == qq:
                    # root term
                    if layer == 1:
                        g = 32 * (wglob % 3)
                        lhs = xaug_s[g:g + 4,
                                     (wglob // 3) * 128:(wglob // 3) * 128 + 128]
                        rr = r1_s[g:g + 4, :]
                    else:
                        lhs = h1T_s[:, wglob * 128:(wglob + 1) * 128]
                        rr = r2_s[:]
                    nc.tensor.matmul(a, lhs, rr, start=False,
                                     stop=(wglob not in last_q),
                                     skip_group_check=True)
                for pi, (k, col) in enumerate(plist):
                    oh = ohp.tile([128, 128], bf16, tag="oh")
                    nc.vector.tensor_scalar(oh[:], iota_s[:],
                                            dsf_s[:, col:col + 1], 0.0,
                                            ALU.is_equal, ALU.bypass)
                    kk = k - ck0h
                    nc.tensor.matmul(a, oh[:],
                                     msg_s[:, 32 * kk:32 * kk + 32],
                                     start=False,
                                     stop=(qq == last_q[wglob]
                                           and pi == len(plist) - 1),
                                     skip_group_check=True)

        def retire(layer, h, aggp, ps, w0, nsub):
            """Retire nsub (<=2) windows starting at global window w0:
            relu -> h1 (layer 1) or relu -> fc1 -> relu -> fc2 -> out."""
            span = 128 * nsub
            trp = ps.tile([32, 256], f32, tag="tr")
            for i in range(nsub):
                wglob = w0 + i
                j = wglob - h * HWN
                a = aggp[:, 32 * j:32 * j + 32]
                hf = pipe.tile([128, 32], f32, tag="hf")
                nc.scalar.activation(hf[:], a, AF.Relu)
                if layer == 1:
                    nc.scalar.activation(
                        h1c_s[:, 32 * wglob:32 * wglob + 32], a, AF.Relu)
                nc.tensor.transpose(trp[:, 128 * i:128 * i + 128],
                                    hf[:], id_s[:])
            if layer == 1:
                nc.scalar.activation(
                    h1T_s[0:32, 128 * w0:128 * w0 + span],
                    trp[:, 0:span], AF.Copy)
            else:
                h2t = pipe.tile([33, 256], f32, tag="h2t")
                nc.scalar.activation(h2t[0:32, 0:span], trp[:, 0:span],
                                     AF.Copy)
                nc.vector.memset(h2t[32:33, 0:span], 1.0)
                f1 = ps.tile([32, 256], f32, tag="f1")
                nc.tensor.matmul(f1[:, 0:span], wf1_s[:], h2t[:, 0:span],
                                 start=True, stop=True)
                h3t = pipe.tile([33, 256], f32, tag="h3t")
                nc.scalar.activation(h3t[0:32, 0:span], f1[:, 0:span],
                                     AF.Relu)
                nc.vector.memset(h3t[32:33, 0:span], 1.0)
                f2 = ps.tile([1, 256], f32, tag="f2")
                nc.tensor.matmul(f2[:, 0:span], wf2_s[:], h3t[:, 0:span],
                                 start=True, stop=True)
                ot = pipe.tile([1, 256], f32, tag="ot")
                nc.scalar.activation(ot[:, 0:span], f2[:, 0:span], AF.Copy)
                nc.sync.dma_start(out=out_d[:, 128 * w0:128 * w0 + span],
                                  in_=ot[:, 0:span])

        def emit_retires(layer, h, aggp, ps):
            """Retire all windows of half h in pairs."""
            w_lo, w_hi = h * HWN, h * HWN + NWH[h]
            for w0 in range(w_lo, w_hi, 2):
                nsub = min(2, w_hi - w0)
                retire(layer, h, aggp, ps, w0, nsub)

        # ================= layer 1 =================
        with tc.tile_pool(name="l1p", bufs=1) as l1p:
            xaug_s = l1p.tile([128, XCOLS], f32, tag="xaug")
            nc.sync.dma_start(out=xaug_s[:], in_=xaug_d[:, :])
            f16_s = l1p.tile([128, CB * 128], bf16, tag="f16")
            ct16_s = l1p.tile([128, CB * 128], bf16, tag="ct16")
            # load and multiply in 4 column pieces
            PIECE = _ceil(CB * 128 // 4, 128)
            for p0 in range(0, CB * 128, PIECE):
                p1 = min(CB * 128, p0 + PIECE)
                nc.sync.dma_start(out=f16_s[:, p0:p1], in_=xs16_d[:, p0:p1])
                nc.sync.dma_start(out=ct16_s[:, p0:p1], in_=ct16_d[:, p0:p1])
                nc.vector.tensor_tensor(f16_s[:, p0:p1], f16_s[:, p0:p1],
                                        ct16_s[:, p0:p1], ALU.mult)

            with nc.named_scope("l1"):
                for h in range(2):
                    ck0h = meta['goff'][h][0] // 128
                    with tc.tile_pool(name="ps1agg", bufs=1,
                                      space="PSUM") as psA, \
                         tc.tile_pool(name="ps1w", bufs=2,
                                      space="PSUM") as psW:
                        aggp = psA.tile([128, HWN * 32], f32, tag="agg")
                        zero_agg(aggp, h)
                        for qq in range(4):
                            # messages for this (h, q) group
                            ck0 = meta['goff'][h][qq] // 128
                            ck1 = ck0 + meta['gsz'][h][qq] // 128
                            for c0 in range(ck0, ck1, 8):
                                c1 = min(ck1, c0 + 8)
                                mp = psW.tile([128, 256], f32, tag="mp")
                                for c in range(c0, c1):
                                    g, b = c % 4, c // 4
                                    nc.tensor.matmul(
                                        mp[:, 32 * (c - c0):32 * (c - c0) + 32],
                                        f16_s[:, 128 * b:128 * b + 128],
                                        V4_s[:, g * HID:(g + 1) * HID],
                                        start=True, stop=True)
                                nc.scalar.activation(
                                    msg_s[:, 32 * (c0 - ck0h):32 * (c1 - ck0h)],
                                    mp[:, 0:32 * (c1 - c0)], AF.Copy)
                            agg_windows(1, h, aggp, qq, ck0h)
                        with tc.tile_pool(name="ps1r", bufs=1,
                                          space="PSUM") as psR:
                            emit_retires(1, h, aggp, psR)

        # ship compact h1, allgather
        nc.sync.dma_start(
            out=cc_in.ap().rearrange("(w p) h -> p w h", p=128),
            in_=h1c_s[:].rearrange("p (w h) -> p w h", h=HID))
        with nc.named_scope("allgather"):
            nc.gpsimd.collective_compute(
                "AllGather", ALU.bypass, replica_groups=[list(range(W))],
                ins=[cc_in.ap().opt()], outs=[cc_out.ap().opt()])

        # ================= layer 2 =================
        with nc.named_scope("l2"), \
             tc.tile_pool(name="cwp", bufs=1) as cwp, \
             tc.tile_pool(name="gtp", bufs=2) as gtp, \
             tc.tile_pool(name="tabp", bufs=2) as tabp:
            for h in range(2):
                e0h = meta['goff'][h][0]
                ck0h = e0h // 128
                with tc.tile_pool(name="ps2agg", bufs=1, space="PSUM") as psA, \
                     tc.tile_pool(name="ps2w", bufs=1, space="PSUM") as psW:
                    aggp = psA.tile([128, HWN * 32], f32, tag="agg")
                    zero_agg(aggp, h)
                    for qq in range(4):
                        # build the gather table for this quarter
                        cw = cwp.tile([128, RANKS_Q * 32], bf16, tag="cw")
                        nc.sync.dma_start(
                            out=cw[:].rearrange("p (r h) -> p r h", h=HID),
                            in_=cc_out.ap().rearrange("(q r p) h -> q p r h",
                                                      q=4, p=128)[qq])
                        tq = tabp.tile([128, RANKS_Q * 128], bf16, tag="tq")
                        tq4 = tq[:].rearrange("p (r d h) -> p r d h", d=4,
                                              h=HID)
                        cw3 = cw[:].rearrange("p (r h) -> p r h", h=HID)
                        for d in range(4):
                            nc.vector.tensor_copy(tq4[:, :, d, :], cw3)
                        for (eoff, csz) in calls[(h, qq)]:
                            Gt = gtp.tile([128, 2048], bf16, tag="G")
                            g3 = Gt[:, 0:csz].rearrange("p (o n) -> p o n",
                                                        o=1)
                            nc.gpsimd.dma_gather(
                                g3, tq[:],
                                gidx_s[:, eoff // 16:(eoff + csz) // 16],
                                csz, csz, 128, transpose=True,
                                single_packet=False,
                                sbuf_tokens_per_rank=128,
                                sbuf_free_dim_per_rank=256,
                                sbuf_free_dim_pad_per_rank=0,
                                sbuf_byte_offset=0)
                            for b in range(csz // 512):
                                t0 = eoff + b * 512
                                ct = pipe.tile([128, 512], bf16, tag="ct")
                                nc.sync.dma_start(out=ct[:],
                                                  in_=ct32_d[:, t0:t0 + 512])
                                Ft = pipe.tile([128, 512], bf16, tag="F")
                                nc.vector.tensor_tensor(
                                    Ft[:], Gt[:, 512 * b:512 * b + 512],
                                    ct[:], ALU.mult)
                                mp = psW.tile([128, 128], f32, tag="mp2")
                                for jj in range(4):
                                    nc.tensor.matmul(
                                        mp[:, 32 * jj:32 * jj + 32],
                                        Ft[:, 128 * jj:128 * jj + 128],
                                        w2_s[:], start=True, stop=True)
                                nc.scalar.activation(
                                    msg_s[:, (t0 - e0h) // 4:
                                          (t0 - e0h) // 4 + 128],
                                    mp[:], AF.Copy)
                        agg_windows(2, h, aggp, qq, ck0h)
                    with tc.tile_pool(name="ps2r", bufs=1,
                                      space="PSUM") as psR:
                        emit_retires(2, h, aggp, psR)
    return nc


def run_kernel(inputs, cfg=None, trace=False):
    cfg = cfg or FULL_CFG
    W = cfg['W']
    params = {k: inputs[k] for k in
              ('Wn1', 'bn1', 'root1', 'b1', 'Wn2', 'bn2', 'root2', 'b2',
               'Wf1', 'bf1', 'Wf2', 'bf2')}
    in_maps, meta = host_prep(inputs['x'], inputs['edge_index'],
                              inputs['edge_attr'], params, cfg)
    nc = build_bass(meta)
    nc.finalize()
    res = run_bass_kernel_spmd(nc, in_maps, core_ids=list(range(W)),
                               trace=trace)
    NLOC = meta['NLOC']
    out = np.zeros((cfg['N'], 1), np.float32)
    for c in range(W):
        out[c * NLOC:(c + 1) * NLOC, 0] = res.results[c]['out'][0, :NLOC]
    return out, res


def kernel(**inputs):
    out, _ = run_kernel(inputs)
    return out


# revision 22
# speedup vs baseline: 2.0075x; 1.4762x over previous
"""NNConv (gnn_message_passing) SPMD kernel for 8 trn2 NeuronCores.

Strategy (dst-sharded, both layers):
  - Each core owns a contiguous range of NLOC nodes (dst sharding). Edges are
    assigned to the core owning their dst, and laid out half-major:
    (dst-window-half, src-quarter, dst-window, dst).
  - Layer 1 needs NO on-device gather: the host ships x[src] and the edge
    coefficients c=[1,ea] replicated into a 16-row outer-product layout
    (pure indexing / replication); one DVE multiply forms F = c (x) x_src and
    one small matmul per 128-edge chunk produces the messages.
  - Layer 2 gathers h1[src] via the SBUF transpose-gather (Q7 SWDGE). All
    compute (messages, one-hot aggregation, root terms, fc) is interleaved
    with the gather stream so the Q7 descriptor generation is the only wall.
  - Aggregation (segment sum over dst): one-hot PE matmuls into a PSUM
    accumulator holding one half of the dst windows (4 banks), accumulated
    across all 4 src-quarters, fused with the root-term matmul and ReLU.
  - One AllGather (compact h1, bf16) between the layers.
  - Edge layout is made identical across cores via a shared R-table
    (cell (half, src-quarter, dst-window) padded to the max count over cores).
"""

import sys

if '/opt/trn_rl_repo' not in sys.path:
    sys.path.insert(0, '/opt/trn_rl_repo')

from contextlib import ExitStack

import ml_dtypes
import numpy as np

import concourse.bacc as bacc
import concourse.bass as bass
from concourse import mybir, tile
from concourse.bass_utils import run_bass_kernel_spmd
from concourse import library_config

BF16 = ml_dtypes.bfloat16
AF = mybir.ActivationFunctionType
ALU = mybir.AluOpType

FULL_CFG = dict(N=100000, E=400000, W=8, DIM=3, HID=32)


def _ceil(a, b):
    return -(-a // b) * b


def make_geom(N, W):
    NLOC = N // W
    NLOCP = _ceil(NLOC, 128)
    NP = W * NLOCP
    assert NP % 4 == 0
    QS = NP // 4          # table rows per src-quarter
    assert QS % 128 == 0
    NW = NLOCP // 128     # dst windows per core
    return NLOC, NLOCP, NP, QS, NW


def wrap_idx16(idx):
    """Edge i -> [i%16, i//16], tiled to 128 partitions (int16)."""
    a = np.asarray(idx, np.int16).reshape(-1, 16).T
    return np.tile(a, (8, 1))


def host_prep(x, edge_index, edge_attr, params, cfg):
    """Build per-core input arrays + shared structural metadata."""
    N, E, W, DIM, HID = cfg['N'], cfg['E'], cfg['W'], cfg['DIM'], cfg['HID']
    NLOC, NLOCP, NP, QS, NW = make_geom(N, W)
    HWN = (NW + 1) // 2                         # windows per half

    src = np.asarray(edge_index[0], np.int64)
    dst = np.asarray(edge_index[1], np.int64)
    ea = np.asarray(edge_attr, np.float32)
    x = np.asarray(x, np.float32)

    tr = (src // NLOC) * NLOCP + (src % NLOC)    # gather-table row
    core = dst // NLOC
    q = tr // QS
    dl = dst % NLOC                              # dst local id
    w = dl // 128                                # dst window
    hf = (w >= HWN).astype(np.int64)             # dst-window half

    # --- shared cell table: R[h, q, w'] = max over cores of cell count -----
    # cells ordered half-major: (h, q, w within half)
    win_in_half = w - hf * HWN
    NWH = [HWN, NW - HWN]                        # windows per half
    key = ((core * 2 + hf) * 4 + q) * HWN + win_in_half
    cnt = np.bincount(key, minlength=W * 2 * 4 * HWN).reshape(W, 2, 4, HWN)
    R = cnt.max(axis=0)                          # [2, 4, HWN]
    # pad each (half, quarter) group total to a multiple of 512
    for h in range(2):
        for qq in range(4):
            tot = int(R[h, qq, :NWH[h]].sum())
            R[h, qq, NWH[h] - 1] += _ceil(max(tot, 512), 512) - tot
    gsz = np.array([[int(R[h, qq, :NWH[h]].sum()) for qq in range(4)]
                    for h in range(2)])          # [2, 4]
    goff = np.zeros((2, 4), np.int64)
    run = 0
    cell_list = []                               # (h, q, w, off, len) in order
    coff = {}
    for h in range(2):
        for qq in range(4):
            goff[h, qq] = run
            for ww in range(NWH[h]):
                wglob = h * HWN + ww
                coff[(h, qq, wglob)] = run
                cell_list.append((h, qq, wglob, run, int(R[h, qq, ww])))
                run += int(R[h, qq, ww])
    EP = run
    assert EP % 512 == 0
    CH = EP // 128                               # chunks

    # --- per-chunk pair metadata (shared) ---------------------------------
    pairs = [[] for _ in range(CH)]              # chunk -> [(wglob, paircol)]
    npairs = 0
    for (h, qq, wglob, off, ln) in cell_list:
        if ln == 0:
            continue
        k0, k1 = off // 128, (off + ln - 1) // 128
        for k in range(k0, k1 + 1):
            pairs[k].append((wglob, npairs))
            npairs += 1
    # cells grouped per (h, q) for emission: [(wglob, [(k, col), ...]), ...]
    cells_hq = {(h, qq): [] for h in range(2) for qq in range(4)}
    for (h, qq, wglob, off, ln) in cell_list:
        plist = []
        if ln > 0:
            k0, k1 = off // 128, (off + ln - 1) // 128
            for k in range(k0, k1 + 1):
                col = next(c for (wv, c) in pairs[k] if wv == wglob)
                plist.append((k, col))
        cells_hq[(h, qq)].append((wglob, plist))
    # last nonempty quarter per window (for matmul stop flags)
    last_q = {}
    first_q = {}
    for h in range(2):
        for qq in range(4):
            for (wglob, plist) in cells_hq[(h, qq)]:
                if plist:
                    last_q[wglob] = qq
                    if wglob not in first_q:
                        first_q[wglob] = qq

    # gather calls per (half, quarter): pieces of <=2048, all 512-multiples
    calls = {(h, qq): [] for h in range(2) for qq in range(4)}
    for h in range(2):
        for qq in range(4):
            o = 0
            while o < gsz[h, qq]:
                s = min(2048, int(gsz[h, qq]) - o)
                calls[(h, qq)].append((int(goff[h, qq]) + o, s))
                o += s

    # --- per-core arrays ---------------------------------------------------
    order = np.lexsort((src, dl, w, q, hf, core))
    gidx = np.zeros((W, EP), np.int64)
    dlv = np.full((W, EP), -10000.0, np.float32)
    cfull = np.zeros((W, 4, EP), np.float32)     # c = [1, ea] per edge slot
    xfull = np.zeros((W, 4, EP), np.float32)     # x[src] (padded) per slot

    so_src = src[order]
    so_tr = tr[order]
    so_core = core[order]
    so_q = q[order]
    so_hf = hf[order]
    so_w = w[order]
    so_dl = dl[order]
    so_ea = ea[order]

    ckey = ((so_core * 2 + so_hf) * 4 + so_q) * NW + so_w
    grp_starts = np.flatnonzero(np.r_[True, ckey[1:] != ckey[:-1]])
    grp_ends = np.r_[grp_starts[1:], len(ckey)]
    for gs, ge in zip(grp_starts, grp_ends):
        c = int(so_core[gs]); hh = int(so_hf[gs])
        qq = int(so_q[gs]); wglob = int(so_w[gs])
        o = coff[(hh, qq, wglob)]
        n = ge - gs
        gidx[c, o:o + n] = so_tr[gs:ge] - qq * QS
        cfull[c, 0, o:o + n] = 1.0
        cfull[c, 1:4, o:o + n] = so_ea[gs:ge].T
        xfull[c, 0:3, o:o + n] = x[so_src[gs:ge]].T
        dlv[c, o:o + n] = so_dl[gs:ge].astype(np.float32)

    # wrapped gather idx [W, 128, EP//16]
    gidx16 = np.stack([wrap_idx16(gidx[c]) for c in range(W)])

    # one-hot scatter tiles, shipped prebuilt: ohs[:, 128*col + j] = 1 iff
    # edge slot p of the pair's chunk has dst offset j within the pair's
    # window. Column order == aggregation emission order.
    ohs = np.zeros((W, 128, max(npairs, 1) * 128), BF16)
    for k in range(CH):
        for (wglob, col) in pairs[k]:
            for c in range(W):
                v = dlv[c, k * 128:(k + 1) * 128] - 128.0 * wglob
                ok = (v >= 0) & (v < 128)
                pp = np.nonzero(ok)[0]
                ohs[c, pp, col * 128 + v[pp].astype(np.int64)] = 1.0

    # --- layer-1 no-gather tensors ----------------------------------------
    # packed [128, ceil(CH/4)*128]: chunk c -> partition group 32*(c%4),
    # columns 128*(c//4); rows within group r = 4*dc + i:
    #   ct16[r] = c_dc[e],  xs16[r] = x_i[src_e] (i<3) else 0
    CB = _ceil(CH, 4) // 4
    ct16 = np.zeros((W, 128, CB * 128), np.float32)
    xs16 = np.zeros((W, 128, CB * 128), np.float32)
    for c in range(W):
        cf = cfull[c]                            # [4, EP]
        xf = xfull[c]                            # [4, EP]
        for g in range(4):
            # chunks with c%4 == g -> columns of block c//4
            ch_ids = np.arange(g, CH, 4)
            colsrc = (ch_ids[:, None] * 128 + np.arange(128)[None, :]).ravel()
            coldst = (np.arange(len(ch_ids))[:, None] * 128 +
                      np.arange(128)[None, :]).ravel()
            for dc in range(4):
                for i in range(4):
                    r = 32 * g + 4 * dc + i
                    ct16[c, r, coldst] = cf[dc, colsrc]
                    if i < 3:
                        xs16[c, r, coldst] = xf[i, colsrc]
    ct16 = ct16.astype(BF16)
    xs16 = xs16.astype(BF16)

    # layer-2 expanded coefficients in DRAM: ct32[32g+i, e] = c_g[e]
    ct32 = np.repeat(cfull, 32, axis=1).astype(BF16)   # [W, 128, EP]

    # --- weights -----------------------------------------------------------
    Wn1 = np.asarray(params['Wn1'], np.float32)
    bn1 = np.asarray(params['bn1'], np.float32)
    Wn2 = np.asarray(params['Wn2'], np.float32)
    bn2 = np.asarray(params['bn2'], np.float32)

    # V4 for layer 1: [128, 4*HID]. Full 128-row contraction per chunk
    # (no PE tiling modes): col-block g holds V16 at rows 32g.., zeros
    # elsewhere, so chunk group g picks out only its rows.
    V16 = np.zeros((32, HID), np.float32)
    B1 = bn1.reshape(DIM, HID)
    W1r = Wn1.reshape(DIM, DIM, HID)
    for dc in range(4):
        for i in range(DIM):
            V16[4 * dc + i] = B1[i] if dc == 0 else W1r[dc - 1, i]
    V4 = np.zeros((128, 4 * HID), np.float32)
    for g in range(4):
        V4[32 * g:32 * g + 32, g * HID:(g + 1) * HID] = V16
    V4 = V4.astype(BF16)

    # w2stack for layer 2: [128, 32]
    w2stack = np.zeros((128, HID), np.float32)
    w2stack[0:HID] = bn2.reshape(HID, HID)
    for d in range(DIM):
        w2stack[32 * (d + 1):32 * (d + 1) + HID] = Wn2[d].reshape(HID, HID)
    w2stack = w2stack.astype(BF16)

    # x_augT packed (per core): window w at [32*(w%3):+4, (w//3)*128:+128]
    XCOLS = _ceil(NW, 3) // 3 * 128
    xaug = np.zeros((W, 128, XCOLS), np.float32)  # cast to bf16 below
    for c in range(W):
        xa = np.zeros((4, NLOCP), np.float32)
        xa[:DIM, :NLOC] = x[c * NLOC:(c + 1) * NLOC].T
        xa[3, :NLOC] = 1.0
        for ww in range(NW):
            xaug[c, 32 * (ww % 3):32 * (ww % 3) + 4,
                 (ww // 3) * 128:(ww // 3) * 128 + 128] = \
                xa[:, ww * 128:(ww + 1) * 128]
    xaug = xaug.astype(BF16)

    root1a = np.concatenate([np.asarray(params['root1'], np.float32),
                             np.asarray(params['b1'], np.float32)[None]], 0)
    r1tri = np.zeros((128, 3 * HID), np.float32)
    for m in range(3):
        r1tri[32 * m:32 * m + 4, m * HID:(m + 1) * HID] = root1a
    r1tri = r1tri.astype(BF16)
    root2a = np.concatenate([np.asarray(params['root2'], np.float32),
                             np.asarray(params['b2'], np.float32)[None]],
                            0).astype(BF16)
    wf1a = np.asarray(params['Wf1'], np.float32).astype(BF16)
    wf2a = np.asarray(params['Wf2'], np.float32).astype(BF16)
    bf1a = np.asarray(params['bf1'], np.float32).reshape(HID, 1)
    ident = np.eye(128, dtype=np.float32)

    CHH = max(int(gsz[0].sum()), int(gsz[1].sum())) // 128
    meta = dict(NLOC=NLOC, NLOCP=NLOCP, NP=NP, QS=QS, NW=NW, HWN=HWN,
                NWH=NWH, EP=EP, CH=CH, CB=CB, CHH=CHH, XCOLS=XCOLS,
                npairs=npairs,
                pairs=pairs, cells_hq=cells_hq, last_q=last_q, first_q=first_q,
                calls=calls, gsz=gsz.tolist(), goff=goff.tolist(),
                W=W, HID=HID, DIM=DIM,
                bf2=float(np.asarray(params['bf2']).ravel()[0]))

    shared = dict(V4=V4, w2stack=w2stack, r1tri=r1tri, root2a=root2a,
                  wf1a=wf1a, wf2a=wf2a, bf1a=bf1a, ident=ident)
    in_maps = []
    for c in range(W):
        m = dict(shared)
        m['gidx'] = gidx16[c]
        m['ohs'] = ohs[c]
        m['xaug'] = xaug[c]
        m['ct16'] = ct16[c]
        m['xs16'] = xs16[c]
        m['ct32'] = ct32[c]
        in_maps.append(m)
    return in_maps, meta


def build_bass(meta):
    W, HID = meta['W'], meta['HID']
    NLOCP, NP, QS, NW = meta['NLOCP'], meta['NP'], meta['QS'], meta['NW']
    HWN, NWH = meta['HWN'], meta['NWH']
    EP, CH, CB, CHH = meta['EP'], meta['CH'], meta['CB'], meta['CHH']
    XCOLS, npairs = meta['XCOLS'], meta['npairs']
    pairs, calls, cells_hq = meta['pairs'], meta['calls'], meta['cells_hq']
    last_q, first_q = meta['last_q'], meta['first_q']
    RANKS_Q = QS // 128
    NWC = NLOCP // 128          # = NW, ranks per core in the table
    f32, bf16, i16 = mybir.dt.float32, mybir.dt.bfloat16, mybir.dt.int16

    nc = bacc.Bacc("TRN2", target_bir_lowering=False, debug=False,
                   num_devices=W, enable_asserts=False)

    # I/O ------------------------------------------------------------------
    gidx_d = nc.dram_tensor("gidx", [128, EP // 16], i16, kind="ExternalInput")
    ohs_d = nc.dram_tensor("ohs", [128, max(npairs, 1) * 128], bf16,
                           kind="ExternalInput")
    xaug_d = nc.dram_tensor("xaug", [128, XCOLS], bf16, kind="ExternalInput")
    ct16_d = nc.dram_tensor("ct16", [128, CB * 128], bf16,
                            kind="ExternalInput")
    xs16_d = nc.dram_tensor("xs16", [128, CB * 128], bf16,
                            kind="ExternalInput")
    ct32_d = nc.dram_tensor("ct32", [128, EP], bf16, kind="ExternalInput")
    V4_d = nc.dram_tensor("V4", [128, 4 * HID], bf16, kind="ExternalInput")
    w2_d = nc.dram_tensor("w2stack", [128, HID], bf16, kind="ExternalInput")
    r1_d = nc.dram_tensor("r1tri", [128, 3 * HID], bf16, kind="ExternalInput")
    r2_d = nc.dram_tensor("root2a", [33, HID], bf16, kind="ExternalInput")
    wf1_d = nc.dram_tensor("wf1a", [HID, HID], bf16, kind="ExternalInput")
    wf2_d = nc.dram_tensor("wf2a", [HID, 1], bf16, kind="ExternalInput")
    bf1_d = nc.dram_tensor("bf1a", [HID, 1], f32, kind="ExternalInput")
    id_d = nc.dram_tensor("ident", [128, 128], f32, kind="ExternalInput")
    out_d = nc.dram_tensor("out", [1, NLOCP], f32, kind="ExternalOutput")
    BF2 = meta['bf2']

    # p-major exchange layout: row (p*NWC + w) holds h1[128*w + p]
    cc_in = nc.dram_tensor("cc_in", [NLOCP, HID], bf16)
    cc_out = nc.dram_tensor("cc_out", [NP, HID], bf16, addr_space="Shared")

    ctx = ExitStack()
    with tile.TileContext(nc) as tc:
      with ctx:
        const = ctx.enter_context(tc.tile_pool(name="const", bufs=1))
        big = ctx.enter_context(tc.tile_pool(name="big", bufs=1))
        pipe = ctx.enter_context(tc.tile_pool(name="pipe", bufs=2))
        ohp = ctx.enter_context(tc.tile_pool(name="ohp", bufs=3))

        nc.gpsimd.load_library(library_config.mlp)

        # ---- constant loads ----
        def load(pool, dram, shape, dtype):
            t = pool.tile(shape, dtype, tag="c_" + dram.name)
            nc.sync.dma_start(out=t[:], in_=dram[:, :])
            return t

        gidx_s = load(const, gidx_d, [128, EP // 16], i16)
        V4_s = load(const, V4_d, [128, 4 * HID], bf16)
        w2_s = load(const, w2_d, [128, HID], bf16)
        r1_s = load(const, r1_d, [128, 3 * HID], bf16)
        r2_s = load(const, r2_d, [33, HID], bf16)
        wf1_s = load(const, wf1_d, [HID, HID], bf16)
        wf2_s = load(const, wf2_d, [HID, 1], bf16)
        bf1_s = load(const, bf1_d, [HID, 1], f32)
        id_s = load(const, id_d, [128, 128], f32)
        zer_s = const.tile([128, 512], bf16, tag="zer")
        nc.vector.memset(zer_s[:], 0.0)

        msg_s = big.tile([128, CHH * 32], bf16)     # msgs of one half
        h1c_s = big.tile([128, NW * 32], bf16)      # compact local h1
        h1T_s = big.tile([33, NLOCP], bf16)         # h1^T augmented
        nc.vector.memset(h1T_s[32:33, :], 1.0)

        # one-hot tile streaming: 16 pairs per [128, 2048] tile, in
        # aggregation emission order (pair col order).
        oh_state = {'tile': None, 'base': 0}

        def oh_lhs(col):
            if oh_state['tile'] is None or col - oh_state['base'] >= 16 \
                    or col < oh_state['base']:
                t = ohp.tile([128, 2048], bf16, tag="oht")
                b = col
                n = min(16, max(npairs, 1) - b)
                nc.sync.dma_start(out=t[:, 0:128 * n],
                                  in_=ohs_d[:, 128 * b:128 * (b + n)])
                oh_state['tile'] = t
                oh_state['base'] = b
            t = oh_state['tile']
            o = (col - oh_state['base']) * 128
            return t[:, o:o + 128]

        def zero_agg(aggp, h):
            """One start=True matmul per 2KB PSUM bank (start marks the whole
            zero-region pending-zero, so interleaved per-window chains must
            all accumulate afterwards with start=False)."""
            tot = NWH[h] * 32
            for off in range(0, tot, 512):
                wd = min(512, tot - off)
                nc.tensor.matmul(aggp[:, off:off + wd], zer_s[:, 0:128],
                                 zer_s[:, 0:wd], start=True, stop=False,
                                 skip_group_check=True)

        def agg_windows(layer, h, aggp, qq, ck0h):
            """Emit aggregation pairs for quarter qq of half h."""
            for (wglob, plist) in cells_hq[(h, qq)]:
                j = wglob - h * HWN
                a = aggp[:, 32 * j:32 * j + 32]
                if first_q.get(wglob, 0) == qq:
                    # root term
                    if layer == 1:
                        m = wglob % 3
                        nc.tensor.matmul(
                            a, xaug_s[:, (wglob // 3) * 128:
                                      (wglob // 3) * 128 + 128],
                            r1_s[:, m * HID:(m + 1) * HID], start=False,
                            stop=(wglob not in last_q),
                            skip_group_check=True)
                    else:
                        nc.tensor.matmul(
                            a, h1T_s[:, wglob * 128:(wglob + 1) * 128],
                            r2_s[:], start=False,
                            stop=(wglob not in last_q),
                            skip_group_check=True)
                for pi, (k, col) in enumerate(plist):
                    kk = k - ck0h
                    nc.tensor.matmul(a, oh_lhs(col),
                                     msg_s[:, 32 * kk:32 * kk + 32],
                                     start=False,
                                     stop=(qq == last_q[wglob]
                                           and pi == len(plist) - 1),
                                     skip_group_check=True)

        def retire(layer, h, aggp, ps, w0, nsub):
            """Retire nsub (<=4) windows starting at global window w0."""
            span = 128 * nsub
            trp = ps.tile([32, 512], f32, tag="tr")
            for i in range(nsub):
                wglob = w0 + i
                j = wglob - h * HWN
                a = aggp[:, 32 * j:32 * j + 32]
                hf = pipe.tile([128, 32], f32, tag="hf")
                nc.scalar.activation(hf[:], a, AF.Relu)
                if layer == 1:
                    nc.scalar.activation(
                        h1c_s[:, 32 * wglob:32 * wglob + 32], a, AF.Relu)
                nc.tensor.transpose(trp[:, 128 * i:128 * i + 128],
                                    hf[:], id_s[:])
            if layer == 1:
                nc.scalar.activation(
                    h1T_s[0:32, 128 * w0:128 * w0 + span],
                    trp[:, 0:span], AF.Copy)
            else:
                h2t = pipe.tile([32, 512], bf16, tag="h2t")
                nc.scalar.activation(h2t[:, 0:span], trp[:, 0:span], AF.Copy)
                f1 = ps.tile([32, 512], f32, tag="f1")
                nc.tensor.matmul(f1[:, 0:span], wf1_s[:], h2t[:, 0:span],
                                 start=True, stop=True)
                h3t = pipe.tile([32, 512], bf16, tag="h3t")
                nc.scalar.activation(h3t[:, 0:span], f1[:, 0:span],
                                     AF.Relu, bias=bf1_s[:, 0:1])
                f2 = ps.tile([1, 512], f32, tag="f2")
                nc.tensor.matmul(f2[:, 0:span], wf2_s[:], h3t[:, 0:span],
                                 start=True, stop=True)
                ot = pipe.tile([1, 512], f32, tag="ot")
                nc.scalar.activation(ot[:, 0:span], f2[:, 0:span], AF.Copy,
                                     bias=BF2)
                nc.sync.dma_start(out=out_d[:, 128 * w0:128 * w0 + span],
                                  in_=ot[:, 0:span])

        def emit_retires(layer, h, aggp, ps):
            w_lo, w_hi = h * HWN, h * HWN + NWH[h]
            for w0 in range(w_lo, w_hi, 4):
                retire(layer, h, aggp, ps, w0, min(4, w_hi - w0))

        # ================= layer 1 =================
        with tc.tile_pool(name="l1p", bufs=1) as l1p:
            xaug_s = l1p.tile([128, XCOLS], bf16, tag="xaug")
            nc.sync.dma_start(out=xaug_s[:], in_=xaug_d[:, :])
            f16_s = l1p.tile([128, CB * 128], bf16, tag="f16")
            ct16_s = l1p.tile([128, CB * 128], bf16, tag="ct16")
            PIECE = _ceil(CB * 128 // 4, 128)
            for p0 in range(0, CB * 128, PIECE):
                p1 = min(CB * 128, p0 + PIECE)
                nc.sync.dma_start(out=f16_s[:, p0:p1], in_=xs16_d[:, p0:p1])
                nc.sync.dma_start(out=ct16_s[:, p0:p1], in_=ct16_d[:, p0:p1])
                nc.vector.tensor_tensor(f16_s[:, p0:p1], f16_s[:, p0:p1],
                                        ct16_s[:, p0:p1], ALU.mult)

            with nc.named_scope("l1"):
                for h in range(2):
                    ck0h = meta['goff'][h][0] // 128
                    with tc.tile_pool(name="ps1agg", bufs=1,
                                      space="PSUM") as psA, \
                         tc.tile_pool(name="ps1w", bufs=2,
                                      space="PSUM") as psW:
                        aggp = psA.tile([128, HWN * 32], f32, tag="agg")
                        zero_agg(aggp, h)
                        for qq in range(4):
                            ck0 = meta['goff'][h][qq] // 128
                            ck1 = ck0 + meta['gsz'][h][qq] // 128
                            for c0 in range(ck0, ck1, 8):
                                c1 = min(ck1, c0 + 8)
                                mp = psW.tile([128, 256], f32, tag="mp")
                                for c in range(c0, c1):
                                    g, b = c % 4, c // 4
                                    nc.tensor.matmul(
                                        mp[:, 32 * (c - c0):32 * (c - c0) + 32],
                                        f16_s[:, 128 * b:128 * b + 128],
                                        V4_s[:, g * HID:(g + 1) * HID],
                                        start=True, stop=True)
                                nc.scalar.activation(
                                    msg_s[:, 32 * (c0 - ck0h):32 * (c1 - ck0h)],
                                    mp[:, 0:32 * (c1 - c0)], AF.Copy)
                            agg_windows(1, h, aggp, qq, ck0h)
                        with tc.tile_pool(name="ps1r", bufs=1,
                                          space="PSUM") as psR:
                            emit_retires(1, h, aggp, psR)

        # ship compact h1 (p-major rows), allgather
        nc.sync.dma_start(
            out=cc_in.ap().rearrange("(p w) h -> p w h", p=128),
            in_=h1c_s[:].rearrange("p (w h) -> p w h", h=HID))
        with nc.named_scope("allgather"):
            nc.gpsimd.collective_compute(
                "AllGather", ALU.bypass, replica_groups=[list(range(W))],
                ins=[cc_in.ap().opt()], outs=[cc_out.ap().opt()])

        # ================= layer 2 =================
        with nc.named_scope("l2"), \
             tc.tile_pool(name="cwp", bufs=1) as cwp, \
             tc.tile_pool(name="gtp", bufs=3) as gtp, \
             tc.tile_pool(name="tabp", bufs=2) as tabp:
            for h in range(2):
                e0h = meta['goff'][h][0]
                ck0h = e0h // 128
                with tc.tile_pool(name="ps2agg", bufs=1, space="PSUM") as psA, \
                     tc.tile_pool(name="ps2w", bufs=1, space="PSUM") as psW:
                    aggp = psA.tile([128, HWN * 32], f32, tag="agg")
                    zero_agg(aggp, h)
                    for qq in range(4):
                        # gather table for this quarter: p-major cc rows make
                        # this a contiguous-run DMA; then 4x replicate on DVE.
                        cw = cwp.tile([128, RANKS_Q * 32], bf16, tag="cw")
                        nc.sync.dma_start(
                            out=cw[:].rearrange("p (c w h) -> p c w h",
                                                c=2, h=HID),
                            in_=cc_out.ap().rearrange(
                                "(cr p w) h -> p cr w h",
                                p=128, w=NWC)[:, 2 * qq:2 * qq + 2])
                        tq = tabp.tile([128, RANKS_Q * 128], bf16, tag="tq")
                        tq4 = tq[:].rearrange("p (r d h) -> p r d h", d=4,
                                              h=HID)
                        cw3 = cw[:].rearrange("p (r h) -> p r h", h=HID)
                        for d in range(4):
                            nc.vector.tensor_copy(tq4[:, :, d, :], cw3)
                        for (eoff, csz) in calls[(h, qq)]:
                            Gt = gtp.tile([128, 2048], bf16, tag="G")
                            g3 = Gt[:, 0:csz].rearrange("p (o n) -> p o n",
                                                        o=1)
                            nc.gpsimd.dma_gather(
                                g3, tq[:],
                                gidx_s[:, eoff // 16:(eoff + csz) // 16],
                                csz, csz, 128, transpose=True,
                                single_packet=False,
                                sbuf_tokens_per_rank=128,
                                sbuf_free_dim_per_rank=256,
                                sbuf_free_dim_pad_per_rank=0,
                                sbuf_byte_offset=0)
                            for b in range(csz // 512):
                                t0 = eoff + b * 512
                                ct = pipe.tile([128, 512], bf16, tag="ct")
                                nc.sync.dma_start(out=ct[:],
                                                  in_=ct32_d[:, t0:t0 + 512])
                                Ft = pipe.tile([128, 512], bf16, tag="F")
                                nc.vector.tensor_tensor(
                                    Ft[:], Gt[:, 512 * b:512 * b + 512],
                                    ct[:], ALU.mult)
                                mp = psW.tile([128, 128], f32, tag="mp2")
                                for jj in range(4):
                                    nc.tensor.matmul(
                                        mp[:, 32 * jj:32 * jj + 32],
                                        Ft[:, 128 * jj:128 * jj + 128],
                                        w2_s[:], start=True, stop=True)
                                nc.scalar.activation(
                                    msg_s[:, (t0 - e0h) // 4:
                                          (t0 - e0h) // 4 + 128],
                                    mp[:], AF.Copy)
                        agg_windows(2, h, aggp, qq, ck0h)
                    with tc.tile_pool(name="ps2r", bufs=1,
                                      space="PSUM") as psR:
                        emit_retires(2, h, aggp, psR)
    return nc


def run_kernel(inputs, cfg=None, trace=False):
    cfg = cfg or FULL_CFG
    W = cfg['W']
    params = {k: inputs[k] for k in
              ('Wn1', 'bn1', 'root1', 'b1', 'Wn2', 'bn2', 'root2', 'b2',
               'Wf1', 'bf1', 'Wf2', 'bf2')}
    in_maps, meta = host_prep(inputs['x'], inputs['edge_index'],
                              inputs['edge_attr'], params, cfg)
    nc = build_bass(meta)
    nc.finalize()
    res = run_bass_kernel_spmd(nc, in_maps, core_ids=list(range(W)),
                               trace=trace)
    NLOC = meta['NLOC']
    out = np.zeros((cfg['N'], 1), np.float32)
    for c in range(W):
        out[c * NLOC:(c + 1) * NLOC, 0] = res.results[c]['out'][0, :NLOC]
    return out, res


def kernel(**inputs):
    out, _ = run_kernel(inputs)
    return out


# revision 23
# speedup vs baseline: 2.1251x; 1.0586x over previous
"""NNConv (gnn_message_passing) SPMD kernel for 8 trn2 NeuronCores.

Strategy (dst-sharded, both layers):
  - Each core owns a contiguous range of NLOC nodes (dst sharding). Edges are
    assigned to the core owning their dst, and laid out half-major:
    (dst-window-half, src-quarter, dst-window, dst).
  - Layer 1 needs NO on-device gather: the host ships x[src] and the edge
    coefficients c=[1,ea] replicated into a 16-row outer-product layout
    (pure indexing / replication); one DVE multiply forms F = c (x) x_src and
    one small matmul per 128-edge chunk produces the messages.
  - Layer 2 gathers h1[src] via the SBUF transpose-gather (Q7 SWDGE). All
    compute (messages, one-hot aggregation, root terms, fc) is interleaved
    with the gather stream so the Q7 descriptor generation is the only wall.
  - Aggregation (segment sum over dst): one-hot PE matmuls into a PSUM
    accumulator holding one half of the dst windows (4 banks), accumulated
    across all 4 src-quarters, fused with the root-term matmul and ReLU.
  - One AllGather (compact h1, bf16) between the layers.
  - Edge layout is made identical across cores via a shared R-table
    (cell (half, src-quarter, dst-window) padded to the max count over cores).
"""

import sys

if '/opt/trn_rl_repo' not in sys.path:
    sys.path.insert(0, '/opt/trn_rl_repo')

from contextlib import ExitStack

import ml_dtypes
import numpy as np

import concourse.bacc as bacc
import concourse.bass as bass
from concourse import mybir, tile
from concourse.bass_utils import run_bass_kernel_spmd
from concourse import library_config

BF16 = ml_dtypes.bfloat16
AF = mybir.ActivationFunctionType
ALU = mybir.AluOpType

FULL_CFG = dict(N=100000, E=400000, W=8, DIM=3, HID=32)


def _ceil(a, b):
    return -(-a // b) * b


def make_geom(N, W):
    NLOC = N // W
    NLOCP = _ceil(NLOC, 128)
    NP = W * NLOCP
    assert NP % 4 == 0
    QS = NP // 4          # table rows per src-quarter
    assert QS % 128 == 0
    NW = NLOCP // 128     # dst windows per core
    return NLOC, NLOCP, NP, QS, NW


def wrap_idx16(idx):
    """Edge i -> [i%16, i//16], tiled to 128 partitions (int16)."""
    a = np.asarray(idx, np.int16).reshape(-1, 16).T
    return np.tile(a, (8, 1))


def host_prep(x, edge_index, edge_attr, params, cfg):
    """Build per-core input arrays + shared structural metadata."""
    N, E, W, DIM, HID = cfg['N'], cfg['E'], cfg['W'], cfg['DIM'], cfg['HID']
    NLOC, NLOCP, NP, QS, NW = make_geom(N, W)
    HWN = (NW + 1) // 2                         # windows per half

    src = np.asarray(edge_index[0], np.int64)
    dst = np.asarray(edge_index[1], np.int64)
    ea = np.asarray(edge_attr, np.float32)
    x = np.asarray(x, np.float32)

    tr = (src // NLOC) * NLOCP + (src % NLOC)    # gather-table row
    core = dst // NLOC
    q = tr // QS
    dl = dst % NLOC                              # dst local id
    w = dl // 128                                # dst window
    hf = (w >= HWN).astype(np.int64)             # dst-window half

    # --- shared cell table: R[h, q, w'] = max over cores of cell count -----
    # cells ordered half-major: (h, q, w within half)
    win_in_half = w - hf * HWN
    NWH = [HWN, NW - HWN]                        # windows per half
    key = ((core * 2 + hf) * 4 + q) * HWN + win_in_half
    cnt = np.bincount(key, minlength=W * 2 * 4 * HWN).reshape(W, 2, 4, HWN)
    R = cnt.max(axis=0)                          # [2, 4, HWN]
    # pad each (half, quarter) group total to a multiple of 512
    for h in range(2):
        for qq in range(4):
            tot = int(R[h, qq, :NWH[h]].sum())
            R[h, qq, NWH[h] - 1] += _ceil(max(tot, 512), 512) - tot
    gsz = np.array([[int(R[h, qq, :NWH[h]].sum()) for qq in range(4)]
                    for h in range(2)])          # [2, 4]
    goff = np.zeros((2, 4), np.int64)
    run = 0
    cell_list = []                               # (h, q, w, off, len) in order
    coff = {}
    for h in range(2):
        for qq in range(4):
            goff[h, qq] = run
            for ww in range(NWH[h]):
                wglob = h * HWN + ww
                coff[(h, qq, wglob)] = run
                cell_list.append((h, qq, wglob, run, int(R[h, qq, ww])))
                run += int(R[h, qq, ww])
    EP = run
    assert EP % 512 == 0
    CH = EP // 128                               # chunks

    # --- per-chunk pair metadata (shared) ---------------------------------
    pairs = [[] for _ in range(CH)]              # chunk -> [(wglob, paircol)]
    npairs = 0
    for (h, qq, wglob, off, ln) in cell_list:
        if ln == 0:
            continue
        k0, k1 = off // 128, (off + ln - 1) // 128
        for k in range(k0, k1 + 1):
            pairs[k].append((wglob, npairs))
            npairs += 1
    # cells grouped per (h, q) for emission: [(wglob, [(k, col), ...]), ...]
    cells_hq = {(h, qq): [] for h in range(2) for qq in range(4)}
    for (h, qq, wglob, off, ln) in cell_list:
        plist = []
        if ln > 0:
            k0, k1 = off // 128, (off + ln - 1) // 128
            for k in range(k0, k1 + 1):
                col = next(c for (wv, c) in pairs[k] if wv == wglob)
                plist.append((k, col))
        cells_hq[(h, qq)].append((wglob, plist))
    # last nonempty quarter per window (for matmul stop flags)
    last_q = {}
    first_q = {}
    for h in range(2):
        for qq in range(4):
            for (wglob, plist) in cells_hq[(h, qq)]:
                if plist:
                    last_q[wglob] = qq
                    if wglob not in first_q:
                        first_q[wglob] = qq

    # gather calls per (half, quarter): pieces of <=2048, all 512-multiples
    calls = {(h, qq): [] for h in range(2) for qq in range(4)}
    for h in range(2):
        for qq in range(4):
            o = 0
            while o < gsz[h, qq]:
                s = min(2048, int(gsz[h, qq]) - o)
                calls[(h, qq)].append((int(goff[h, qq]) + o, s))
                o += s

    # --- per-core arrays ---------------------------------------------------
    order = np.lexsort((src, dl, w, q, hf, core))
    gidx = np.zeros((W, EP), np.int64)
    dlv = np.full((W, EP), -10000.0, np.float32)
    cfull = np.zeros((W, 4, EP), np.float32)     # c = [1, ea] per edge slot
    xfull = np.zeros((W, 4, EP), np.float32)     # x[src] (padded) per slot

    so_src = src[order]
    so_tr = tr[order]
    so_core = core[order]
    so_q = q[order]
    so_hf = hf[order]
    so_w = w[order]
    so_dl = dl[order]
    so_ea = ea[order]

    ckey = ((so_core * 2 + so_hf) * 4 + so_q) * NW + so_w
    grp_starts = np.flatnonzero(np.r_[True, ckey[1:] != ckey[:-1]])
    grp_ends = np.r_[grp_starts[1:], len(ckey)]
    for gs, ge in zip(grp_starts, grp_ends):
        c = int(so_core[gs]); hh = int(so_hf[gs])
        qq = int(so_q[gs]); wglob = int(so_w[gs])
        o = coff[(hh, qq, wglob)]
        n = ge - gs
        gidx[c, o:o + n] = so_tr[gs:ge] - qq * QS
        cfull[c, 0, o:o + n] = 1.0
        cfull[c, 1:4, o:o + n] = so_ea[gs:ge].T
        xfull[c, 0:3, o:o + n] = x[so_src[gs:ge]].T
        dlv[c, o:o + n] = so_dl[gs:ge].astype(np.float32)

    # wrapped gather idx [W, 128, EP//16]
    gidx16 = np.stack([wrap_idx16(gidx[c]) for c in range(W)])

    # one-hot scatter tiles, shipped prebuilt: ohs[:, 128*col + j] = 1 iff
    # edge slot p of the pair's chunk has dst offset j within the pair's
    # window. Column order == aggregation emission order.
    ohs = np.zeros((W, 128, max(npairs, 1) * 128), ml_dtypes.float8_e4m3fn)
    for k in range(CH):
        for (wglob, col) in pairs[k]:
            for c in range(W):
                v = dlv[c, k * 128:(k + 1) * 128] - 128.0 * wglob
                ok = (v >= 0) & (v < 128)
                pp = np.nonzero(ok)[0]
                ohs[c, pp, col * 128 + v[pp].astype(np.int64)] = 1.0

    # --- layer-1 no-gather tensors ----------------------------------------
    # packed [128, ceil(CH/4)*128]: chunk c -> partition group 32*(c%4),
    # columns 128*(c//4); rows within group r = 4*dc + i:
    #   ct16[r] = c_dc[e],  xs16[r] = x_i[src_e] (i<3) else 0
    CB = _ceil(CH, 4) // 4
    ct16 = np.zeros((W, 128, CB * 128), np.float32)
    xs16 = np.zeros((W, 128, CB * 128), np.float32)
    for c in range(W):
        cf = cfull[c]                            # [4, EP]
        xf = xfull[c]                            # [4, EP]
        for g in range(4):
            # chunks with c%4 == g -> columns of block c//4
            ch_ids = np.arange(g, CH, 4)
            colsrc = (ch_ids[:, None] * 128 + np.arange(128)[None, :]).ravel()
            coldst = (np.arange(len(ch_ids))[:, None] * 128 +
                      np.arange(128)[None, :]).ravel()
            for dc in range(4):
                for i in range(4):
                    r = 32 * g + 4 * dc + i
                    ct16[c, r, coldst] = cf[dc, colsrc]
                    if i < 3:
                        xs16[c, r, coldst] = xf[i, colsrc]
    ct16 = ct16.astype(BF16)
    xs16 = xs16.astype(BF16)

    # layer-2 expanded coefficients in DRAM: ct32[32g+i, e] = c_g[e]
    ct32 = np.repeat(cfull, 32, axis=1).astype(BF16)   # [W, 128, EP]

    # --- weights -----------------------------------------------------------
    Wn1 = np.asarray(params['Wn1'], np.float32)
    bn1 = np.asarray(params['bn1'], np.float32)
    Wn2 = np.asarray(params['Wn2'], np.float32)
    bn2 = np.asarray(params['bn2'], np.float32)

    # V4 for layer 1: [128, 4*HID]. Full 128-row contraction per chunk
    # (no PE tiling modes): col-block g holds V16 at rows 32g.., zeros
    # elsewhere, so chunk group g picks out only its rows.
    V16 = np.zeros((32, HID), np.float32)
    B1 = bn1.reshape(DIM, HID)
    W1r = Wn1.reshape(DIM, DIM, HID)
    for dc in range(4):
        for i in range(DIM):
            V16[4 * dc + i] = B1[i] if dc == 0 else W1r[dc - 1, i]
    V4 = np.zeros((128, 4 * HID), np.float32)
    for g in range(4):
        V4[32 * g:32 * g + 32, g * HID:(g + 1) * HID] = V16
    V4 = V4.astype(BF16)

    # w2stack for layer 2: [128, 32]
    w2stack = np.zeros((128, HID), np.float32)
    w2stack[0:HID] = bn2.reshape(HID, HID)
    for d in range(DIM):
        w2stack[32 * (d + 1):32 * (d + 1) + HID] = Wn2[d].reshape(HID, HID)
    w2stack = w2stack.astype(BF16)

    # x_augT packed (per core): window w at [32*(w%3):+4, (w//3)*128:+128]
    XCOLS = _ceil(NW, 3) // 3 * 128
    xaug = np.zeros((W, 128, XCOLS), np.float32)  # cast to bf16 below
    for c in range(W):
        xa = np.zeros((4, NLOCP), np.float32)
        xa[:DIM, :NLOC] = x[c * NLOC:(c + 1) * NLOC].T
        xa[3, :NLOC] = 1.0
        for ww in range(NW):
            xaug[c, 32 * (ww % 3):32 * (ww % 3) + 4,
                 (ww // 3) * 128:(ww // 3) * 128 + 128] = \
                xa[:, ww * 128:(ww + 1) * 128]
    xaug = xaug.astype(BF16)

    root1a = np.concatenate([np.asarray(params['root1'], np.float32),
                             np.asarray(params['b1'], np.float32)[None]], 0)
    r1tri = np.zeros((128, 3 * HID), np.float32)
    for m in range(3):
        r1tri[32 * m:32 * m + 4, m * HID:(m + 1) * HID] = root1a
    r1tri = r1tri.astype(BF16)
    root2a = np.concatenate([np.asarray(params['root2'], np.float32),
                             np.asarray(params['b2'], np.float32)[None]],
                            0).astype(BF16)
    wf1a = np.asarray(params['Wf1'], np.float32).astype(BF16)
    wf2a = np.asarray(params['Wf2'], np.float32).astype(BF16)
    bf1a = np.asarray(params['bf1'], np.float32).reshape(HID, 1)
    ident = np.eye(128, dtype=np.float32)

    CHH = max(int(gsz[0].sum()), int(gsz[1].sum())) // 128
    meta = dict(NLOC=NLOC, NLOCP=NLOCP, NP=NP, QS=QS, NW=NW, HWN=HWN,
                NWH=NWH, EP=EP, CH=CH, CB=CB, CHH=CHH, XCOLS=XCOLS,
                npairs=npairs,
                pairs=pairs, cells_hq=cells_hq, last_q=last_q, first_q=first_q,
                calls=calls, gsz=gsz.tolist(), goff=goff.tolist(),
                W=W, HID=HID, DIM=DIM,
                bf2=float(np.asarray(params['bf2']).ravel()[0]))

    shared = dict(V4=V4, w2stack=w2stack, r1tri=r1tri, root2a=root2a,
                  wf1a=wf1a, wf2a=wf2a, bf1a=bf1a, ident=ident)
    in_maps = []
    for c in range(W):
        m = dict(shared)
        m['gidx'] = gidx16[c]
        m['ohs'] = ohs[c]
        m['xaug'] = xaug[c]
        m['ct16'] = ct16[c]
        m['xs16'] = xs16[c]
        m['ct32'] = ct32[c]
        in_maps.append(m)
    return in_maps, meta


def build_bass(meta):
    W, HID = meta['W'], meta['HID']
    NLOCP, NP, QS, NW = meta['NLOCP'], meta['NP'], meta['QS'], meta['NW']
    HWN, NWH = meta['HWN'], meta['NWH']
    EP, CH, CB, CHH = meta['EP'], meta['CH'], meta['CB'], meta['CHH']
    XCOLS, npairs = meta['XCOLS'], meta['npairs']
    pairs, calls, cells_hq = meta['pairs'], meta['calls'], meta['cells_hq']
    last_q, first_q = meta['last_q'], meta['first_q']
    RANKS_Q = QS // 128
    NWC = NLOCP // 128          # = NW, ranks per core in the table
    f32, bf16, i16 = mybir.dt.float32, mybir.dt.bfloat16, mybir.dt.int16
    fp8 = mybir.dt.float8e4

    nc = bacc.Bacc("TRN2", target_bir_lowering=False, debug=False,
                   num_devices=W, enable_asserts=False)

    # I/O ------------------------------------------------------------------
    gidx_d = nc.dram_tensor("gidx", [128, EP // 16], i16, kind="ExternalInput")
    ohs_d = nc.dram_tensor("ohs", [128, max(npairs, 1) * 128], fp8,
                           kind="ExternalInput")
    xaug_d = nc.dram_tensor("xaug", [128, XCOLS], bf16, kind="ExternalInput")
    ct16_d = nc.dram_tensor("ct16", [128, CB * 128], bf16,
                            kind="ExternalInput")
    xs16_d = nc.dram_tensor("xs16", [128, CB * 128], bf16,
                            kind="ExternalInput")
    ct32_d = nc.dram_tensor("ct32", [128, EP], bf16, kind="ExternalInput")
    V4_d = nc.dram_tensor("V4", [128, 4 * HID], bf16, kind="ExternalInput")
    w2_d = nc.dram_tensor("w2stack", [128, HID], bf16, kind="ExternalInput")
    r1_d = nc.dram_tensor("r1tri", [128, 3 * HID], bf16, kind="ExternalInput")
    r2_d = nc.dram_tensor("root2a", [33, HID], bf16, kind="ExternalInput")
    wf1_d = nc.dram_tensor("wf1a", [HID, HID], bf16, kind="ExternalInput")
    wf2_d = nc.dram_tensor("wf2a", [HID, 1], bf16, kind="ExternalInput")
    bf1_d = nc.dram_tensor("bf1a", [HID, 1], f32, kind="ExternalInput")
    id_d = nc.dram_tensor("ident", [128, 128], f32, kind="ExternalInput")
    out_d = nc.dram_tensor("out", [1, NLOCP], f32, kind="ExternalOutput")
    BF2 = meta['bf2']

    # p-major exchange layout: row (p*NWC + w) holds h1[128*w + p]
    cc_in = nc.dram_tensor("cc_in", [NLOCP, HID], bf16)
    cc_out = nc.dram_tensor("cc_out", [NP, HID], bf16, addr_space="Shared")

    ctx = ExitStack()
    with tile.TileContext(nc) as tc:
      with ctx:
        const = ctx.enter_context(tc.tile_pool(name="const", bufs=1))
        big = ctx.enter_context(tc.tile_pool(name="big", bufs=1))
        pipe = ctx.enter_context(tc.tile_pool(name="pipe", bufs=2))
        ohp = ctx.enter_context(tc.tile_pool(name="ohp", bufs=3))

        nc.gpsimd.load_library(library_config.mlp)

        # ---- constant loads ----
        def load(pool, dram, shape, dtype):
            t = pool.tile(shape, dtype, tag="c_" + dram.name)
            nc.sync.dma_start(out=t[:], in_=dram[:, :])
            return t

        gidx_s = load(const, gidx_d, [128, EP // 16], i16)
        V4_s = load(const, V4_d, [128, 4 * HID], bf16)
        w2_s = load(const, w2_d, [128, HID], bf16)
        r1_s = load(const, r1_d, [128, 3 * HID], bf16)
        r2_s = load(const, r2_d, [33, HID], bf16)
        wf1_s = load(const, wf1_d, [HID, HID], bf16)
        wf2_s = load(const, wf2_d, [HID, 1], bf16)
        bf1_s = load(const, bf1_d, [HID, 1], f32)
        id_s = load(const, id_d, [128, 128], f32)
        zer_s = const.tile([128, 512], bf16, tag="zer")
        nc.vector.memset(zer_s[:], 0.0)

        msg_s = big.tile([128, CHH * 32], bf16)     # msgs of one half
        h1c_s = big.tile([128, NW * 32], bf16)      # compact local h1
        h1T_s = big.tile([33, NLOCP], bf16)         # h1^T augmented
        nc.vector.memset(h1T_s[32:33, :], 1.0)

        # one-hot tile streaming: 32 pairs per [128, 4096] fp8 tile, in
        # aggregation emission order; loads alternate between the SP and ACT
        # HWDGE queues so the stream is never one-queue-latency-bound.
        oh_state = {'tile': None, 'base': 0, 'n': 0}

        def oh_lhs(col):
            if oh_state['tile'] is None or col - oh_state['base'] >= 32 \
                    or col < oh_state['base']:
                t = ohp.tile([128, 4096], fp8, tag="oht")
                b = col
                n = min(32, max(npairs, 1) - b)
                eng = nc.sync if oh_state['n'] % 2 == 0 else nc.scalar
                eng.dma_start(out=t[:, 0:128 * n],
                              in_=ohs_d[:, 128 * b:128 * (b + n)])
                oh_state['tile'] = t
                oh_state['base'] = b
                oh_state['n'] += 1
            t = oh_state['tile']
            o = (col - oh_state['base']) * 128
            return t[:, o:o + 128]

        def zero_agg(aggp, h):
            """One start=True matmul per 2KB PSUM bank (start marks the whole
            zero-region pending-zero, so interleaved per-window chains must
            all accumulate afterwards with start=False)."""
            tot = NWH[h] * 32
            for off in range(0, tot, 512):
                wd = min(512, tot - off)
                nc.tensor.matmul(aggp[:, off:off + wd], zer_s[:, 0:128],
                                 zer_s[:, 0:wd], start=True, stop=False,
                                 skip_group_check=True)

        def agg_windows(layer, h, aggp, qq, ck0h):
            """Emit aggregation pairs for quarter qq of half h."""
            for (wglob, plist) in cells_hq[(h, qq)]:
                j = wglob - h * HWN
                a = aggp[:, 32 * j:32 * j + 32]
                if first_q.get(wglob, 0) == qq:
                    # root term
                    if layer == 1:
                        m = wglob % 3
                        nc.tensor.matmul(
                            a, xaug_s[:, (wglob // 3) * 128:
                                      (wglob // 3) * 128 + 128],
                            r1_s[:, m * HID:(m + 1) * HID], start=False,
                            stop=(wglob not in last_q),
                            skip_group_check=True)
                    else:
                        nc.tensor.matmul(
                            a, h1T_s[:, wglob * 128:(wglob + 1) * 128],
                            r2_s[:], start=False,
                            stop=(wglob not in last_q),
                            skip_group_check=True)
                for pi, (k, col) in enumerate(plist):
                    kk = k - ck0h
                    nc.tensor.matmul(a, oh_lhs(col),
                                     msg_s[:, 32 * kk:32 * kk + 32],
                                     start=False,
                                     stop=(qq == last_q[wglob]
                                           and pi == len(plist) - 1),
                                     skip_group_check=True)

        def retire(layer, h, aggp, ps, w0, nsub):
            """Retire nsub (<=4) windows starting at global window w0."""
            span = 128 * nsub
            trp = ps.tile([32, 512], f32, tag="tr")
            for i in range(nsub):
                wglob = w0 + i
                j = wglob - h * HWN
                a = aggp[:, 32 * j:32 * j + 32]
                hf = pipe.tile([128, 32], f32, tag="hf")
                nc.scalar.activation(hf[:], a, AF.Relu)
                if layer == 1:
                    nc.scalar.activation(
                        h1c_s[:, 32 * wglob:32 * wglob + 32], a, AF.Relu)
                nc.tensor.transpose(trp[:, 128 * i:128 * i + 128],
                                    hf[:], id_s[:])
            if layer == 1:
                nc.scalar.activation(
                    h1T_s[0:32, 128 * w0:128 * w0 + span],
                    trp[:, 0:span], AF.Copy)
            else:
                h2t = pipe.tile([32, 512], bf16, tag="h2t")
                nc.scalar.activation(h2t[:, 0:span], trp[:, 0:span], AF.Copy)
                f1 = ps.tile([32, 512], f32, tag="f1")
                nc.tensor.matmul(f1[:, 0:span], wf1_s[:], h2t[:, 0:span],
                                 start=True, stop=True)
                h3t = pipe.tile([32, 512], bf16, tag="h3t")
                nc.scalar.activation(h3t[:, 0:span], f1[:, 0:span],
                                     AF.Relu, bias=bf1_s[:, 0:1])
                f2 = ps.tile([1, 512], f32, tag="f2")
                nc.tensor.matmul(f2[:, 0:span], wf2_s[:], h3t[:, 0:span],
                                 start=True, stop=True)
                ot = pipe.tile([1, 512], f32, tag="ot")
                nc.scalar.activation(ot[:, 0:span], f2[:, 0:span], AF.Copy,
                                     bias=BF2)
                nc.sync.dma_start(out=out_d[:, 128 * w0:128 * w0 + span],
                                  in_=ot[:, 0:span])

        def emit_retires(layer, h, aggp, ps):
            w_lo, w_hi = h * HWN, h * HWN + NWH[h]
            for w0 in range(w_lo, w_hi, 4):
                retire(layer, h, aggp, ps, w0, min(4, w_hi - w0))

        # ================= layer 1 =================
        with tc.tile_pool(name="l1p", bufs=1) as l1p:
            xaug_s = l1p.tile([128, XCOLS], bf16, tag="xaug")
            nc.sync.dma_start(out=xaug_s[:], in_=xaug_d[:, :])
            f16_s = l1p.tile([128, CB * 128], bf16, tag="f16")
            ct16_s = l1p.tile([128, CB * 128], bf16, tag="ct16")
            PIECE = _ceil(CB * 128 // 4, 128)
            for p0 in range(0, CB * 128, PIECE):
                p1 = min(CB * 128, p0 + PIECE)
                nc.sync.dma_start(out=f16_s[:, p0:p1], in_=xs16_d[:, p0:p1])
                nc.sync.dma_start(out=ct16_s[:, p0:p1], in_=ct16_d[:, p0:p1])
                nc.vector.tensor_tensor(f16_s[:, p0:p1], f16_s[:, p0:p1],
                                        ct16_s[:, p0:p1], ALU.mult)

            with nc.named_scope("l1"):
                for h in range(2):
                    ck0h = meta['goff'][h][0] // 128
                    with tc.tile_pool(name="ps1agg", bufs=1,
                                      space="PSUM") as psA, \
                         tc.tile_pool(name="ps1w", bufs=2,
                                      space="PSUM") as psW:
                        aggp = psA.tile([128, HWN * 32], f32, tag="agg")
                        zero_agg(aggp, h)
                        for qq in range(4):
                            ck0 = meta['goff'][h][qq] // 128
                            ck1 = ck0 + meta['gsz'][h][qq] // 128
                            for c0 in range(ck0, ck1, 8):
                                c1 = min(ck1, c0 + 8)
                                mp = psW.tile([128, 256], f32, tag="mp")
                                for c in range(c0, c1):
                                    g, b = c % 4, c // 4
                                    nc.tensor.matmul(
                                        mp[:, 32 * (c - c0):32 * (c - c0) + 32],
                                        f16_s[:, 128 * b:128 * b + 128],
                                        V4_s[:, g * HID:(g + 1) * HID],
                                        start=True, stop=True)
                                nc.vector.tensor_copy(
                                    msg_s[:, 32 * (c0 - ck0h):32 * (c1 - ck0h)],
                                    mp[:, 0:32 * (c1 - c0)])
                            agg_windows(1, h, aggp, qq, ck0h)
                        with tc.tile_pool(name="ps1r", bufs=1,
                                          space="PSUM") as psR:
                            emit_retires(1, h, aggp, psR)

        # ship compact h1 (p-major rows), allgather
        nc.sync.dma_start(
            out=cc_in.ap().rearrange("(p w) h -> p w h", p=128),
            in_=h1c_s[:].rearrange("p (w h) -> p w h", h=HID))
        with nc.named_scope("allgather"):
            nc.gpsimd.collective_compute(
                "AllGather", ALU.bypass, replica_groups=[list(range(W))],
                ins=[cc_in.ap().opt()], outs=[cc_out.ap().opt()])

        # ================= layer 2 =================
        with nc.named_scope("l2"), \
             tc.tile_pool(name="cwp", bufs=1) as cwp, \
             tc.tile_pool(name="gtp", bufs=3) as gtp, \
             tc.tile_pool(name="tabp", bufs=2) as tabp:
            for h in range(2):
                e0h = meta['goff'][h][0]
                ck0h = e0h // 128
                with tc.tile_pool(name="ps2agg", bufs=1, space="PSUM") as psA, \
                     tc.tile_pool(name="ps2w", bufs=1, space="PSUM") as psW:
                    aggp = psA.tile([128, HWN * 32], f32, tag="agg")
                    zero_agg(aggp, h)
                    for qq in range(4):
                        # gather table for this quarter: p-major cc rows make
                        # this a contiguous-run DMA; then 4x replicate on DVE.
                        cw = cwp.tile([128, RANKS_Q * 32], bf16, tag="cw")
                        nc.sync.dma_start(
                            out=cw[:].rearrange("p (c w h) -> p c w h",
                                                c=2, h=HID),
                            in_=cc_out.ap().rearrange(
                                "(cr p w) h -> p cr w h",
                                p=128, w=NWC)[:, 2 * qq:2 * qq + 2])
                        tq = tabp.tile([128, RANKS_Q * 128], bf16, tag="tq")
                        tq4 = tq[:].rearrange("p (r d h) -> p r d h", d=4,
                                              h=HID)
                        cw3 = cw[:].rearrange("p (r h) -> p r h", h=HID)
                        for d in range(4):
                            nc.vector.tensor_copy(tq4[:, :, d, :], cw3)
                        for (eoff, csz) in calls[(h, qq)]:
                            Gt = gtp.tile([128, 2048], bf16, tag="G")
                            g3 = Gt[:, 0:csz].rearrange("p (o n) -> p o n",
                                                        o=1)
                            nc.gpsimd.dma_gather(
                                g3, tq[:],
                                gidx_s[:, eoff // 16:(eoff + csz) // 16],
                                csz, csz, 128, transpose=True,
                                single_packet=False,
                                sbuf_tokens_per_rank=128,
                                sbuf_free_dim_per_rank=256,
                                sbuf_free_dim_pad_per_rank=0,
                                sbuf_byte_offset=0)
                            for b in range(csz // 512):
                                t0 = eoff + b * 512
                                ct = pipe.tile([128, 512], bf16, tag="ct")
                                nc.sync.dma_start(out=ct[:],
                                                  in_=ct32_d[:, t0:t0 + 512])
                                Ft = pipe.tile([128, 512], bf16, tag="F")
                                nc.vector.tensor_tensor(
                                    Ft[:], Gt[:, 512 * b:512 * b + 512],
                                    ct[:], ALU.mult)
                                mp = psW.tile([128, 128], f32, tag="mp2")
                                for jj in range(4):
                                    nc.tensor.matmul(
                                        mp[:, 32 * jj:32 * jj + 32],
                                        Ft[:, 128 * jj:128 * jj + 128],
                                        w2_s[:], start=True, stop=True)
                                nc.scalar.activation(
                                    msg_s[:, (t0 - e0h) // 4:
                                          (t0 - e0h) // 4 + 128],
                                    mp[:], AF.Copy)
                        agg_windows(2, h, aggp, qq, ck0h)
                    with tc.tile_pool(name="ps2r", bufs=1,
                                      space="PSUM") as psR:
                        emit_retires(2, h, aggp, psR)
    return nc


def run_kernel(inputs, cfg=None, trace=False):
    cfg = cfg or FULL_CFG
    W = cfg['W']
    params = {k: inputs[k] for k in
              ('Wn1', 'bn1', 'root1', 'b1', 'Wn2', 'bn2', 'root2', 'b2',
               'Wf1', 'bf1', 'Wf2', 'bf2')}
    in_maps, meta = host_prep(inputs['x'], inputs['edge_index'],
                              inputs['edge_attr'], params, cfg)
    nc = build_bass(meta)
    nc.finalize()
    res = run_bass_kernel_spmd(nc, in_maps, core_ids=list(range(W)),
                               trace=trace)
    NLOC = meta['NLOC']
    out = np.zeros((cfg['N'], 1), np.float32)
    for c in range(W):
        out[c * NLOC:(c + 1) * NLOC, 0] = res.results[c]['out'][0, :NLOC]
    return out, res


def kernel(**inputs):
    out, _ = run_kernel(inputs)
    return out


# revision 29
# speedup vs baseline: 2.1515x; 1.0124x over previous
"""NNConv (gnn_message_passing) SPMD kernel for 8 trn2 NeuronCores.

Strategy (dst-sharded, both layers):
  - Each core owns a contiguous range of NLOC nodes (dst sharding). Edges are
    assigned to the core owning their dst, and laid out half-major:
    (dst-window-half, src-quarter, dst-window, dst).
  - Layer 1 needs NO on-device gather: the host ships x[src] and the edge
    coefficients c=[1,ea] replicated into a 16-row outer-product layout
    (pure indexing / replication); one DVE multiply forms F = c (x) x_src and
    one small matmul per 128-edge chunk produces the messages.
  - Layer 2 gathers h1[src] via the SBUF transpose-gather (Q7 SWDGE). All
    compute (messages, one-hot aggregation, root terms, fc) is interleaved
    with the gather stream so the Q7 descriptor generation is the only wall.
  - Aggregation (segment sum over dst): one-hot PE matmuls into a PSUM
    accumulator holding one half of the dst windows (4 banks), accumulated
    across all 4 src-quarters, fused with the root-term matmul and ReLU.
  - One AllGather (compact h1, bf16) between the layers.
  - Edge layout is made identical across cores via a shared R-table
    (cell (half, src-quarter, dst-window) padded to the max count over cores).
"""

import sys

if '/opt/trn_rl_repo' not in sys.path:
    sys.path.insert(0, '/opt/trn_rl_repo')

from contextlib import ExitStack

import ml_dtypes
import numpy as np

import concourse.bacc as bacc
import concourse.bass as bass
from concourse import mybir, tile
from concourse.bass_utils import run_bass_kernel_spmd
from concourse import library_config

BF16 = ml_dtypes.bfloat16
AF = mybir.ActivationFunctionType
ALU = mybir.AluOpType

FULL_CFG = dict(N=100000, E=400000, W=8, DIM=3, HID=32)


def _ceil(a, b):
    return -(-a // b) * b


def make_geom(N, W):
    NLOC = N // W
    NLOCP = _ceil(NLOC, 128)
    NP = W * NLOCP
    assert NP % 4 == 0
    QS = NP // 4          # table rows per src-quarter
    assert QS % 128 == 0
    NW = NLOCP // 128     # dst windows per core
    return NLOC, NLOCP, NP, QS, NW


def wrap_idx16(idx):
    """Edge i -> [i%16, i//16], tiled to 128 partitions (int16)."""
    a = np.asarray(idx, np.int16).reshape(-1, 16).T
    return np.tile(a, (8, 1))


def host_prep(x, edge_index, edge_attr, params, cfg):
    """Build per-core input arrays + shared structural metadata."""
    N, E, W, DIM, HID = cfg['N'], cfg['E'], cfg['W'], cfg['DIM'], cfg['HID']
    NLOC, NLOCP, NP, QS, NW = make_geom(N, W)
    HWN = (NW + 1) // 2                         # dst windows per half
    NWB = NW - HWN
    SHN = HWN * 128                             # nodes per src half

    src = np.asarray(edge_index[0], np.int64)
    dst = np.asarray(edge_index[1], np.int64)
    ea = np.asarray(edge_attr, np.float32)
    x = np.asarray(x, np.float32)

    core = dst // NLOC
    scr = src // NLOC                            # src-owning core
    snl = src % NLOC                             # src local id
    q = scr // 2                                 # src quarter (core pair)
    sh = (snl // SHN).astype(np.int64)           # src window-half
    dl = dst % NLOC
    w = dl // 128                                # dst window
    hf = (w >= HWN).astype(np.int64)             # dst-window half

    # --- shared cell table: groups (h, q, sh), cells (h, q, sh, w') --------
    win_in_half = w - hf * HWN
    NWH = [HWN, NWB]
    key = (((core * 2 + hf) * 4 + q) * 2 + sh) * HWN + win_in_half
    cnt = np.bincount(key, minlength=W * 2 * 4 * 2 * HWN)
    cnt = cnt.reshape(W, 2, 4, 2, HWN)
    R = cnt.max(axis=0)                          # [2, 4, 2, HWN]
    for h in range(2):
        for qq in range(4):
            for ss in range(2):
                tot = int(R[h, qq, ss, :NWH[h]].sum())
                R[h, qq, ss, NWH[h] - 1] += _ceil(max(tot, 512), 128) - tot
    gsz = np.zeros((2, 4, 2), np.int64)
    goff = np.zeros((2, 4, 2), np.int64)
    run = 0
    cell_list = []                       # (h, q, sh, wglob, off, len)
    coff = {}
    # group order == emission order: (h, sh-major, then quarter)
    for h in range(2):
        for ss in range(2):
            for qq in range(4):
                goff[h, qq, ss] = run
                gsz[h, qq, ss] = int(R[h, qq, ss, :NWH[h]].sum())
                for ww in range(NWH[h]):
                    wglob = h * HWN + ww
                    coff[(h, qq, ss, wglob)] = run
                    cell_list.append((h, qq, ss, wglob, run,
                                      int(R[h, qq, ss, ww])))
                    run += int(R[h, qq, ss, ww])
    EP = run
    assert EP % 128 == 0
    CH = EP // 128

    # --- per-chunk pair metadata (shared) ---------------------------------
    pairs = [[] for _ in range(CH)]
    npairs = 0
    for (h, qq, ss, wglob, off, ln) in cell_list:
        if ln == 0:
            continue
        k0, k1 = off // 128, (off + ln - 1) // 128
        for k in range(k0, k1 + 1):
            pairs[k].append((wglob, npairs))
            npairs += 1
    # cells grouped per (h, q, sh) in emission order
    cells_g = {(h, qq, ss): [] for h in range(2) for qq in range(4)
               for ss in range(2)}
    for (h, qq, ss, wglob, off, ln) in cell_list:
        plist = []
        if ln > 0:
            k0, k1 = off // 128, (off + ln - 1) // 128
            for k in range(k0, k1 + 1):
                col = next(c for (wv, c) in pairs[k] if wv == wglob)
                plist.append((k, col))
        cells_g[(h, qq, ss)].append((wglob, plist))
    # emission sequence within a half: s_idx = ss*4 + qq  (sh-major)
    first_s = {}
    last_s = {}
    for ss in range(2):
        for qq in range(4):
            si = ss * 4 + qq
            for h in range(2):
                for (wglob, plist) in cells_g[(h, qq, ss)]:
                    if plist:
                        last_s[wglob] = si
                        if wglob not in first_s:
                            first_s[wglob] = si

    # gather calls per (h, q, sh): <=2048 pieces; final group tapers
    calls = {}
    for h in range(2):
        for qq in range(4):
            for ss in range(2):
                lst = []
                o = 0
                left = int(gsz[h, qq, ss])
                while o < left:
                    rem = left - o
                    sz = min(2048, rem)
                    if h == 1 and qq == 3 and ss == 1 and 512 < rem <= 2048:
                        sz = 512 if rem % 512 == 0 else rem % 512
                    lst.append((int(goff[h, qq, ss]) + o, sz))
                    o += sz
                calls[(h, qq, ss)] = lst

    # --- per-core arrays ---------------------------------------------------
    order = np.lexsort((src, dl, w, sh, q, hf, core))
    gidx = np.zeros((W, EP), np.int64)
    dlv = np.full((W, EP), -10000.0, np.float32)
    cfull = np.zeros((W, 4, EP), np.float32)
    xfull = np.zeros((W, 4, EP), np.float32)

    so_src = src[order]
    so_core = core[order]
    so_q = q[order]
    so_sh = sh[order]
    so_hf = hf[order]
    so_w = w[order]
    so_dl = dl[order]
    so_ea = ea[order]
    so_scr = scr[order]
    so_snl = snl[order]

    ckey = (((so_core * 2 + so_hf) * 4 + so_q) * 2 + so_sh) * NW + so_w
    grp_starts = np.flatnonzero(np.r_[True, ckey[1:] != ckey[:-1]])
    grp_ends = np.r_[grp_starts[1:], len(ckey)]
    for gs, ge in zip(grp_starts, grp_ends):
        c = int(so_core[gs]); hh = int(so_hf[gs])
        qq = int(so_q[gs]); ss = int(so_sh[gs]); wglob = int(so_w[gs])
        o = coff[(hh, qq, ss, wglob)]
        n = ge - gs
        # table token index within the (q, sh) table:
        # rank = (scr%2)*HWN + (snl - sh*SHN)//128, token = snl%128
        gidx[c, o:o + n] = ((so_scr[gs:ge] % 2) * SHN +
                            (so_snl[gs:ge] - ss * SHN))
        cfull[c, 0, o:o + n] = 1.0
        cfull[c, 1:4, o:o + n] = so_ea[gs:ge].T
        xfull[c, 0:3, o:o + n] = x[so_src[gs:ge]].T
        dlv[c, o:o + n] = so_dl[gs:ge].astype(np.float32)

    gidx16 = np.stack([wrap_idx16(gidx[c]) for c in range(W)])

    # one-hot scatter tiles (fp8), columns in aggregation emission order
    ohs = np.zeros((W, 128, max(npairs, 1) * 128), ml_dtypes.float8_e4m3fn)
    for k in range(CH):
        for (wglob, col) in pairs[k]:
            for c in range(W):
                v = dlv[c, k * 128:(k + 1) * 128] - 128.0 * wglob
                ok = (v >= 0) & (v < 128)
                pp = np.nonzero(ok)[0]
                ohs[c, pp, col * 128 + v[pp].astype(np.int64)] = 1.0

    # --- layer-1 no-gather tensors ----------------------------------------
    CB = _ceil(CH, 4) // 4
    ct16 = np.zeros((W, 128, CB * 128), np.float32)
    xs16 = np.zeros((W, 128, CB * 128), np.float32)
    for c in range(W):
        cf = cfull[c]
        xf = xfull[c]
        for g in range(4):
            ch_ids = np.arange(g, CH, 4)
            colsrc = (ch_ids[:, None] * 128 + np.arange(128)[None, :]).ravel()
            coldst = (np.arange(len(ch_ids))[:, None] * 128 +
                      np.arange(128)[None, :]).ravel()
            for dc in range(4):
                for i in range(4):
                    r = 32 * g + 4 * dc + i
                    ct16[c, r, coldst] = cf[dc, colsrc]
                    if i < 3:
                        xs16[c, r, coldst] = xf[i, colsrc]
    ct16 = ct16.astype(BF16)
    xs16 = xs16.astype(BF16)

    ct32 = np.repeat(cfull, 32, axis=1).astype(BF16)   # [W, 128, EP]

    # --- weights -----------------------------------------------------------
    Wn1 = np.asarray(params['Wn1'], np.float32)
    bn1 = np.asarray(params['bn1'], np.float32)
    Wn2 = np.asarray(params['Wn2'], np.float32)
    bn2 = np.asarray(params['bn2'], np.float32)

    V16 = np.zeros((32, HID), np.float32)
    B1 = bn1.reshape(DIM, HID)
    W1r = Wn1.reshape(DIM, DIM, HID)
    for dc in range(4):
        for i in range(DIM):
            V16[4 * dc + i] = B1[i] if dc == 0 else W1r[dc - 1, i]
    V4 = np.zeros((128, 4 * HID), np.float32)
    for g in range(4):
        V4[32 * g:32 * g + 32, g * HID:(g + 1) * HID] = V16
    V4 = V4.astype(BF16)

    w2stack = np.zeros((128, HID), np.float32)
    w2stack[0:HID] = bn2.reshape(HID, HID)
    for d in range(DIM):
        w2stack[32 * (d + 1):32 * (d + 1) + HID] = Wn2[d].reshape(HID, HID)
    w2stack = w2stack.astype(BF16)

    XCOLS = _ceil(NW, 3) // 3 * 128
    xaug = np.zeros((W, 128, XCOLS), np.float32)
    for c in range(W):
        xa = np.zeros((4, NLOCP), np.float32)
        xa[:DIM, :NLOC] = x[c * NLOC:(c + 1) * NLOC].T
        xa[3, :NLOC] = 1.0
        for ww in range(NW):
            xaug[c, 32 * (ww % 3):32 * (ww % 3) + 4,
                 (ww // 3) * 128:(ww // 3) * 128 + 128] = \
                xa[:, ww * 128:(ww + 1) * 128]
    xaug = xaug.astype(BF16)

    root1a = np.concatenate([np.asarray(params['root1'], np.float32),
                             np.asarray(params['b1'], np.float32)[None]], 0)
    r1tri = np.zeros((128, 3 * HID), np.float32)
    for m in range(3):
        r1tri[32 * m:32 * m + 4, m * HID:(m + 1) * HID] = root1a
    r1tri = r1tri.astype(BF16)
    root2a = np.concatenate([np.asarray(params['root2'], np.float32),
                             np.asarray(params['b2'], np.float32)[None]],
                            0).astype(BF16)
    wf1a = np.asarray(params['Wf1'], np.float32).astype(BF16)
    wf2a = np.asarray(params['Wf2'], np.float32).astype(BF16)
    bf1a = np.asarray(params['bf1'], np.float32).reshape(HID, 1)
    ident = np.eye(128, dtype=np.float32)

    CHH = max(int(gsz[0].sum()), int(gsz[1].sum())) // 128
    meta = dict(NLOC=NLOC, NLOCP=NLOCP, NP=NP, QS=QS, NW=NW, HWN=HWN,
                NWB=NWB, SHN=SHN, NWH=NWH, EP=EP, CH=CH, CB=CB, CHH=CHH,
                XCOLS=XCOLS, npairs=npairs,
                pairs=pairs, cells_g=cells_g, last_s=last_s, first_s=first_s,
                calls=calls, gsz=gsz.tolist(), goff=goff.tolist(),
                W=W, HID=HID, DIM=DIM,
                bf2=float(np.asarray(params['bf2']).ravel()[0]))

    shared = dict(V4=V4, w2stack=w2stack, r1tri=r1tri, root2a=root2a,
                  wf1a=wf1a, wf2a=wf2a, bf1a=bf1a, ident=ident)
    in_maps = []
    for c in range(W):
        m = dict(shared)
        m['gidx'] = gidx16[c]
        m['ohs'] = ohs[c]
        m['xaug'] = xaug[c]
        m['ct16'] = ct16[c]
        m['xs16'] = xs16[c]
        m['ct32'] = ct32[c]
        in_maps.append(m)
    return in_maps, meta


def build_bass(meta):
    W, HID = meta['W'], meta['HID']
    NLOCP, NP, QS, NW = meta['NLOCP'], meta['NP'], meta['QS'], meta['NW']
    HWN, NWB, SHN = meta['HWN'], meta['NWB'], meta['SHN']
    NWH = meta['NWH']
    EP, CH, CB, CHH = meta['EP'], meta['CH'], meta['CB'], meta['CHH']
    XCOLS, npairs = meta['XCOLS'], meta['npairs']
    pairs, calls, cells_g = meta['pairs'], meta['calls'], meta['cells_g']
    last_s, first_s = meta['last_s'], meta['first_s']
    RANKS_SH = 2 * HWN            # ranks per (quarter, src-half) table
    f32, bf16, i16 = mybir.dt.float32, mybir.dt.bfloat16, mybir.dt.int16
    fp8 = mybir.dt.float8e4

    nc = bacc.Bacc("TRN2", target_bir_lowering=False, debug=False,
                   num_devices=W, enable_asserts=False)

    # I/O ------------------------------------------------------------------
    gidx_d = nc.dram_tensor("gidx", [128, EP // 16], i16, kind="ExternalInput")
    ohs_d = nc.dram_tensor("ohs", [128, max(npairs, 1) * 128], fp8,
                           kind="ExternalInput")
    xaug_d = nc.dram_tensor("xaug", [128, XCOLS], bf16, kind="ExternalInput")
    ct16_d = nc.dram_tensor("ct16", [128, CB * 128], bf16,
                            kind="ExternalInput")
    xs16_d = nc.dram_tensor("xs16", [128, CB * 128], bf16,
                            kind="ExternalInput")
    ct32_d = nc.dram_tensor("ct32", [128, EP], bf16, kind="ExternalInput")
    V4_d = nc.dram_tensor("V4", [128, 4 * HID], bf16, kind="ExternalInput")
    w2_d = nc.dram_tensor("w2stack", [128, HID], bf16, kind="ExternalInput")
    r1_d = nc.dram_tensor("r1tri", [128, 3 * HID], bf16, kind="ExternalInput")
    r2_d = nc.dram_tensor("root2a", [33, HID], bf16, kind="ExternalInput")
    wf1_d = nc.dram_tensor("wf1a", [HID, HID], bf16, kind="ExternalInput")
    wf2_d = nc.dram_tensor("wf2a", [HID, 1], bf16, kind="ExternalInput")
    bf1_d = nc.dram_tensor("bf1a", [HID, 1], f32, kind="ExternalInput")
    id_d = nc.dram_tensor("ident", [128, 128], f32, kind="ExternalInput")
    out_d = nc.dram_tensor("out", [1, NLOCP], f32, kind="ExternalOutput")
    BF2 = meta['bf2']

    # p-major exchange layout, split by dst-window half: each collective
    # fires as soon as its half of layer 1 retires, and the matching
    # src-half gathers of layer 2 start right after it.
    cc_in1 = nc.dram_tensor("cc_in1", [128 * HWN, HID], bf16)
    cc_in2 = nc.dram_tensor("cc_in2", [128 * NWB, HID], bf16)
    cc_out1 = nc.dram_tensor("cc_out1", [W * 128 * HWN, HID], bf16,
                             addr_space="Shared")
    cc_out2 = nc.dram_tensor("cc_out2", [W * 128 * NWB, HID], bf16,
                             addr_space="Shared")

    ctx = ExitStack()
    with tile.TileContext(nc) as tc:
      with ctx:
        const = ctx.enter_context(tc.tile_pool(name="const", bufs=1))
        big = ctx.enter_context(tc.tile_pool(name="big", bufs=1))
        pipe = ctx.enter_context(tc.tile_pool(name="pipe", bufs=2))
        ohp = ctx.enter_context(tc.tile_pool(name="ohp", bufs=3))

        nc.gpsimd.load_library(library_config.mlp)

        def load(pool, dram, shape, dtype):
            t = pool.tile(shape, dtype, tag="c_" + dram.name)
            nc.sync.dma_start(out=t[:], in_=dram[:, :])
            return t

        gidx_s = load(const, gidx_d, [128, EP // 16], i16)
        V4_s = load(const, V4_d, [128, 4 * HID], bf16)
        w2_s = load(const, w2_d, [128, HID], bf16)
        r1_s = load(const, r1_d, [128, 3 * HID], bf16)
        r2_s = load(const, r2_d, [33, HID], bf16)
        wf1_s = load(const, wf1_d, [HID, HID], bf16)
        wf2_s = load(const, wf2_d, [HID, 1], bf16)
        bf1_s = load(const, bf1_d, [HID, 1], f32)
        id_s = load(const, id_d, [128, 128], f32)
        zer_s = const.tile([128, 512], bf16, tag="zer")
        nc.vector.memset(zer_s[:], 0.0)

        msg_s = big.tile([128, CHH * 32], bf16)
        h1c_a = big.tile([128, HWN * 32], bf16)
        h1c_b = big.tile([128, NWB * 32], bf16)
        h1T_s = big.tile([33, NLOCP], bf16)
        nc.vector.memset(h1T_s[32:33, :], 1.0)

        # one-hot tile streaming: 32 pairs per [128, 4096] fp8 tile,
        # loads alternating between the SP and ACT HWDGE queues.
        oh_state = {'tile': None, 'base': 0, 'n': 0}

        def oh_lhs(col):
            if oh_state['tile'] is None or col - oh_state['base'] >= 32 \
                    or col < oh_state['base']:
                t = ohp.tile([128, 4096], fp8, tag="oht")
                b = col
                n = min(32, max(npairs, 1) - b)
                eng = nc.sync if oh_state['n'] % 2 == 0 else nc.scalar
                eng.dma_start(out=t[:, 0:128 * n],
                              in_=ohs_d[:, 128 * b:128 * (b + n)])
                oh_state['tile'] = t
                oh_state['base'] = b
                oh_state['n'] += 1
            t = oh_state['tile']
            o = (col - oh_state['base']) * 128
            return t[:, o:o + 128]

        def zero_agg(aggp, h):
            """One start=True matmul per PSUM bank; everything after
            accumulates with start=False (start marks the whole 2KB
            zero-region pending-zero)."""
            tot = NWH[h] * 32
            for off in range(0, tot, 512):
                wd = min(512, tot - off)
                nc.tensor.matmul(aggp[:, off:off + wd], zer_s[:, 0:128],
                                 zer_s[:, 0:wd], start=True, stop=False,
                                 skip_group_check=True)

        def agg_windows(layer, h, aggp, qq, ss, ck0h):
            """Emit aggregation pairs for group (h, qq, ss)."""
            si = ss * 4 + qq
            for (wglob, plist) in cells_g[(h, qq, ss)]:
                j = wglob - h * HWN
                a = aggp[:, 32 * j:32 * j + 32]
                if first_s.get(wglob, 0) == si:
                    if layer == 1:
                        m = wglob % 3
                        nc.tensor.matmul(
                            a, xaug_s[:, (wglob // 3) * 128:
                                      (wglob // 3) * 128 + 128],
                            r1_s[:, m * HID:(m + 1) * HID], start=False,
                            stop=(wglob not in last_s),
                            skip_group_check=True)
                    else:
                        nc.tensor.matmul(
                            a, h1T_s[:, wglob * 128:(wglob + 1) * 128],
                            r2_s[:], start=False,
                            stop=(wglob not in last_s),
                            skip_group_check=True)
                for pi, (k, col) in enumerate(plist):
                    kk = k - ck0h
                    nc.tensor.matmul(a, oh_lhs(col),
                                     msg_s[:, 32 * kk:32 * kk + 32],
                                     start=False,
                                     stop=(si == last_s[wglob]
                                           and pi == len(plist) - 1),
                                     skip_group_check=True)

        def retire(layer, h, aggp, ps, w0, nsub):
            span = 128 * nsub
            trp = ps.tile([32, 512], f32, tag="tr")
            for i in range(nsub):
                wglob = w0 + i
                j = wglob - h * HWN
                a = aggp[:, 32 * j:32 * j + 32]
                hf = pipe.tile([128, 32], f32, tag="hf")
                nc.scalar.activation(hf[:], a, AF.Relu)
                if layer == 1:
                    if wglob < HWN:
                        nc.scalar.activation(
                            h1c_a[:, 32 * wglob:32 * wglob + 32], a, AF.Relu)
                    else:
                        nc.scalar.activation(
                            h1c_b[:, 32 * (wglob - HWN):
                                  32 * (wglob - HWN) + 32], a, AF.Relu)
                nc.tensor.transpose(trp[:, 128 * i:128 * i + 128],
                                    hf[:], id_s[:])
            if layer == 1:
                nc.scalar.activation(
                    h1T_s[0:32, 128 * w0:128 * w0 + span],
                    trp[:, 0:span], AF.Copy)
            else:
                h2t = pipe.tile([32, 512], bf16, tag="h2t")
                nc.scalar.activation(h2t[:, 0:span], trp[:, 0:span], AF.Copy)
                f1 = ps.tile([32, 512], f32, tag="f1")
                nc.tensor.matmul(f1[:, 0:span], wf1_s[:], h2t[:, 0:span],
                                 start=True, stop=True)
                h3t = pipe.tile([32, 512], bf16, tag="h3t")
                nc.scalar.activation(h3t[:, 0:span], f1[:, 0:span],
                                     AF.Relu, bias=bf1_s[:, 0:1])
                f2 = ps.tile([1, 512], f32, tag="f2")
                nc.tensor.matmul(f2[:, 0:span], wf2_s[:], h3t[:, 0:span],
                                 start=True, stop=True)
                ot = pipe.tile([1, 512], f32, tag="ot")
                nc.scalar.activation(ot[:, 0:span], f2[:, 0:span], AF.Copy,
                                     bias=BF2)
                nc.sync.dma_start(out=out_d[:, 128 * w0:128 * w0 + span],
                                  in_=ot[:, 0:span])

        def emit_retires(layer, h, aggp, ps):
            w_lo, w_hi = h * HWN, h * HWN + NWH[h]
            for w0 in range(w_lo, w_hi, 4):
                retire(layer, h, aggp, ps, w0, min(4, w_hi - w0))

        # ================= layer 1 =================
        with tc.tile_pool(name="l1p", bufs=1) as l1p:
            xaug_s = l1p.tile([128, XCOLS], bf16, tag="xaug")
            nc.sync.dma_start(out=xaug_s[:], in_=xaug_d[:, :])
            f16_s = l1p.tile([128, CB * 128], bf16, tag="f16")
            ct16_s = l1p.tile([128, CB * 128], bf16, tag="ct16")
            PIECE = _ceil(CB * 128 // 4, 128)
            for p0 in range(0, CB * 128, PIECE):
                p1 = min(CB * 128, p0 + PIECE)
                nc.sync.dma_start(out=f16_s[:, p0:p1], in_=xs16_d[:, p0:p1])
                nc.sync.dma_start(out=ct16_s[:, p0:p1], in_=ct16_d[:, p0:p1])
                nc.vector.tensor_tensor(f16_s[:, p0:p1], f16_s[:, p0:p1],
                                        ct16_s[:, p0:p1], ALU.mult)

            with nc.named_scope("l1"):
                for h in range(2):
                    ck0h = meta['goff'][h][0][0] // 128
                    with tc.tile_pool(name="ps1agg", bufs=1,
                                      space="PSUM") as psA, \
                         tc.tile_pool(name="ps1w", bufs=2,
                                      space="PSUM") as psW:
                        aggp = psA.tile([128, HWN * 32], f32, tag="agg")
                        zero_agg(aggp, h)
                        for ss in range(2):
                            for qq in range(4):
                                ck0 = meta['goff'][h][qq][ss] // 128
                                ck1 = ck0 + meta['gsz'][h][qq][ss] // 128
                                for c0 in range(ck0, ck1, 8):
                                    c1 = min(ck1, c0 + 8)
                                    mp = psW.tile([128, 256], f32, tag="mp")
                                    for c in range(c0, c1):
                                        g, b = c % 4, c // 4
                                        nc.tensor.matmul(
                                            mp[:, 32 * (c - c0):
                                               32 * (c - c0) + 32],
                                            f16_s[:, 128 * b:128 * b + 128],
                                            V4_s[:, g * HID:(g + 1) * HID],
                                            start=True, stop=True)
                                    nc.vector.tensor_copy(
                                        msg_s[:, 32 * (c0 - ck0h):
                                              32 * (c1 - ck0h)],
                                        mp[:, 0:32 * (c1 - c0)])
                                agg_windows(1, h, aggp, qq, ss, ck0h)
                        with tc.tile_pool(name="ps1r", bufs=1,
                                          space="PSUM") as psR:
                            emit_retires(1, h, aggp, psR)

        # ship compact h1 (p-major rows), allgather per half
        nc.sync.dma_start(
            out=cc_in1.ap().rearrange("(p w) h -> p w h", p=128),
            in_=h1c_a[:].rearrange("p (w h) -> p w h", h=HID))
        with nc.named_scope("allgather"):
            nc.gpsimd.collective_compute(
                "AllGather", ALU.bypass, replica_groups=[list(range(W))],
                ins=[cc_in1.ap().opt()], outs=[cc_out1.ap().opt()])
        nc.sync.dma_start(
            out=cc_in2.ap().rearrange("(p w) h -> p w h", p=128),
            in_=h1c_b[:].rearrange("p (w h) -> p w h", h=HID))
        with nc.named_scope("allgather2"):
            nc.gpsimd.collective_compute(
                "AllGather", ALU.bypass, replica_groups=[list(range(W))],
                ins=[cc_in2.ap().opt()], outs=[cc_out2.ap().opt()])

        # ================= layer 2 =================
        with nc.named_scope("l2"), \
             tc.tile_pool(name="cwp", bufs=1) as cwp, \
             tc.tile_pool(name="gtp", bufs=3) as gtp, \
             tc.tile_pool(name="tabp", bufs=4) as tabp:
            for h in range(2):
                e0h = meta['goff'][h][0][0]
                ck0h = e0h // 128
                with tc.tile_pool(name="ps2agg", bufs=1, space="PSUM") as psA, \
                     tc.tile_pool(name="ps2w", bufs=1, space="PSUM") as psW:
                    aggp = psA.tile([128, HWN * 32], f32, tag="agg")
                    zero_agg(aggp, h)
                    for ss in range(2):
                        cc_sh = cc_out1 if ss == 0 else cc_out2
                        WSH = HWN if ss == 0 else NWB
                        for qq in range(4):
                            # gather table for (quarter, src-half)
                            cw = cwp.tile([128, RANKS_SH * 32], bf16,
                                          tag="cw")
                            nc.sync.dma_start(
                                out=cw[:].rearrange("p (c w h) -> p c w h",
                                                    c=2, h=HID),
                                in_=cc_sh.ap().rearrange(
                                    "(cr p w) h -> p cr w h",
                                    p=128, w=WSH)[:, 2 * qq:2 * qq + 2])
                            tq = tabp.tile([128, RANKS_SH * 128], bf16,
                                           tag="tq")
                            tq4 = tq[:].rearrange("p (r d h) -> p r d h",
                                                  d=4, h=HID)
                            cw3 = cw[:].rearrange("p (r h) -> p r h", h=HID)
                            for d in range(4):
                                nc.vector.tensor_copy(tq4[:, :, d, :], cw3)
                            for (eoff, csz) in calls[(h, qq, ss)]:
                                Gt = gtp.tile([128, 2048], bf16, tag="G")
                                g3 = Gt[:, 0:csz].rearrange(
                                    "p (o n) -> p o n", o=1)
                                nc.gpsimd.dma_gather(
                                    g3, tq[:],
                                    gidx_s[:, eoff // 16:(eoff + csz) // 16],
                                    csz, csz, 128, transpose=True,
                                    single_packet=False,
                                    sbuf_tokens_per_rank=128,
                                    sbuf_free_dim_per_rank=256,
                                    sbuf_free_dim_pad_per_rank=0,
                                    sbuf_byte_offset=0)
                                for b in range(_ceil(csz, 512) // 512):
                                    t0 = eoff + b * 512
                                    bw = min(512, csz - b * 512)
                                    nch = bw // 128
                                    ct = pipe.tile([128, 512], bf16,
                                                   tag="ct")
                                    nc.sync.dma_start(
                                        out=ct[:, 0:bw],
                                        in_=ct32_d[:, t0:t0 + bw])
                                    Ft = pipe.tile([128, 512], bf16, tag="F")
                                    nc.vector.tensor_tensor(
                                        Ft[:, 0:bw],
                                        Gt[:, 512 * b:512 * b + bw],
                                        ct[:, 0:bw], ALU.mult)
                                    mp = psW.tile([128, 128], f32, tag="mp2")
                                    for jj in range(nch):
                                        nc.tensor.matmul(
                                            mp[:, 32 * jj:32 * jj + 32],
                                            Ft[:, 128 * jj:128 * jj + 128],
                                            w2_s[:], start=True, stop=True)
                                    nc.scalar.activation(
                                        msg_s[:, (t0 - e0h) // 4:
                                              (t0 - e0h) // 4 + 32 * nch],
                                        mp[:, 0:32 * nch], AF.Copy)
                            agg_windows(2, h, aggp, qq, ss, ck0h)
                    with tc.tile_pool(name="ps2r", bufs=1,
                                      space="PSUM") as psR:
                        emit_retires(2, h, aggp, psR)
    return nc


def run_kernel(inputs, cfg=None, trace=False):
    cfg = cfg or FULL_CFG
    W = cfg['W']
    params = {k: inputs[k] for k in
              ('Wn1', 'bn1', 'root1', 'b1', 'Wn2', 'bn2', 'root2', 'b2',
               'Wf1', 'bf1', 'Wf2', 'bf2')}
    in_maps, meta = host_prep(inputs['x'], inputs['edge_index'],
                              inputs['edge_attr'], params, cfg)
    nc = build_bass(meta)
    nc.finalize()
    res = run_bass_kernel_spmd(nc, in_maps, core_ids=list(range(W)),
                               trace=trace)
    NLOC = meta['NLOC']
    out = np.zeros((cfg['N'], 1), np.float32)
    for c in range(W):
        out[c * NLOC:(c + 1) * NLOC, 0] = res.results[c]['out'][0, :NLOC]
    return out, res


def kernel(**inputs):
    out, _ = run_kernel(inputs)
    return out
